# revision 57
# baseline (speedup 1.0000x reference)
"""Causal self-attention (B=2, T=2048, D=1024, H=16) on 8 TRN2 NeuronCores.

Sharding: data-parallel over batch (2) x tensor-parallel over head groups (4).
Each core handles 1 batch x 4 heads: Wq/Wk/Wv column-sharded, Wo row-sharded;
each core emits a partial (T, D) output and the host sums 4 partials per batch.

v2 design (vs the fp32r baseline):
  - x is transposed on the HOST and shipped as bf16 [P, QS, DC, 512]
    slab-major, eliminating all 128 PE transposes and their PSUM evictions.
  - All matmul operands are bf16 (fp32 PSUM accumulation): enables the PE's
    fast-weight-load path (fp32r blocks FWL), halves eviction/DMA bytes.
  - No mask-inject matmuls: diagonal S tiles compute only columns [d, 512);
    exp is restricted to the written PSUM region (split calls on diagonal
    k-pairs), and the in-tile causal triangle is applied post-exp as a bf16
    tensor_mul with a host-provided [128,128] upper-triangular mask.
  - All PSUM evictions run on DVE; the scalar engine does (almost) only exp.
  - Softmax denominators ride in V' ones-columns (parity layout: even heads
    col 64 / rows 0-63 data, odd heads col 0 / rows 64-127 data); per (qs,h)
    the denom row is folded via SBUF DMA, reciprocal'd on DVE, unfolded and
    rank-1-broadcast on PE, then fused into the oT eviction multiply.

v3 changes (163.0us -> 152.4us), all pipeline/startup/drain plumbing:
  - V' padding is generated on-device (pad-strip + ones-column memsets)
    instead of DMAing a 2MB host constant; the big zero-fills are split
    gpsimd/vector so the vector queue is free for the first qkv evictions
    (the DVE preamble pile-up was gating the whole qs=0 phase).
  - Startup DMA rings rebalanced: x slab 0 chunked on sync, wq/wk/tri on
    scalar, wv/wo on gpsimd, x slab 1 split across both HWDGE rings (it was
    landing at ~26us on the overloaded scalar ring, stalling qs=1).
  - proj/yout/bcast PSUM moved off the spair ring into a dedicated 2-bank
    ring (their allocations serialized behind spair's exp readers at every
    slab boundary); spair drops to 2 bufs.
  - y DMAs issue per 512-column half on sync+gpsimd queues mid-kernel
    (keeping ~600ns dispatches off the scalar/exp engine) and sync+scalar
    in the drain (where scalar is idle).
  - x slab 0 ships as 8 single-dc chunks; the qs=0 projections chase its
    arrival (~14us, chip-HBM-bound: all 8 cores pull startup data at once)
    and finer chunks keep each stall under the HAM idle window.
  - Drain y tiles 12/13 pre-open their fc0 matmuls on the idle spair/pj
    rings (high_priority) to span the final normB chain; the drain normB
    bcast rides o_ps; tiles 14/15 follow on o_ps.
  - qs=3 runs heads in order [1,0,3,2] so the final normB is the cheap
    PE-broadcast path, and y-tile-12's fc0 matmuls are pre-opened
    (high_priority) to keep the PE warm through the final norm chain.

Explored and rejected (measured on HW via probe kernel): 64-row-tiled S
matmuls give a true 2x when paired back-to-back (108.8 ns per 512-col MM),
but interleaving 64-row and 128-row tile modes costs ~105ns per mode switch,
erasing the win at the per-step granularity PSUM allows (8 banks cap the S
burst depth at 2). fp8 attention fails the 2e-2 gate (~4-12% output noise).
"""

import sys, os, types

sys.path.insert(0, "/opt/trn_rl_repo")

import numpy as np
from contextlib import ExitStack

import concourse.bass as bass
import concourse.mybir as mybir
import concourse.tile as tile
from concourse import bacc

B, T, D, H = 2, 2048, 1024, 16
DH = D // H          # 64
NCORES = 8
HG = 4               # heads per core
F = HG * DH          # 256 local features per core
P = 128
F32 = mybir.dt.float32
F32R = mybir.dt.float32r
BF16 = mybir.dt.bfloat16
FP8 = mybir.dt.float8e4

TT = T // P          # 16 t-tiles
QS = T // 512        # 4 q-slabs
DC = D // P          # 8 d-chunks

LAST_RESULTS = None  # BassKernelResults of the most recent hardware run


def _install_ntff_hook():
    if "antenv.axon_hooks" in sys.modules:
        return
    try:
        import antenv
        from trn_agent_boot.trn_boot import _ntff_profile_via_ctypes

        m = types.ModuleType("antenv.axon_hooks")
        h = _ntff_profile_via_ctypes("/opt/axon/libaxon_pjrt.so")
        m.get_axon_ntff_profile_hook = lambda: h
        m.set_axon_ntff_profile_hook = lambda hh: None
        sys.modules["antenv.axon_hooks"] = m
        antenv.axon_hooks = m
    except Exception:
        pass


def build_nc():
    nc = bacc.Bacc("TRN2", target_bir_lowering=False, debug=False)

    # x^T slab-major: [P, QS, DC, 512] so one q-slab is a contiguous
    # 8KB-per-partition DMA run
    xt_d = nc.dram_tensor("xt", [P, QS, DC, 512], BF16, kind="ExternalInput").ap()
    wq_d = nc.dram_tensor("wq", [P, DC, F], BF16, kind="ExternalInput").ap()
    wk_d = nc.dram_tensor("wk", [P, DC, F], BF16, kind="ExternalInput").ap()
    wv_d = nc.dram_tensor("wv", [P, DC, F], BF16, kind="ExternalInput").ap()
    wo_d = nc.dram_tensor("wo", [P, 2, D], BF16, kind="ExternalInput").ap()
    tri_d = nc.dram_tensor("tri", [P, P], BF16, kind="ExternalInput").ap()
    y_d = nc.dram_tensor("y", [T, D], BF16, kind="ExternalOutput").ap()

    with tile.TileContext(nc) as tc, ExitStack() as ctx:
        const = ctx.enter_context(tc.tile_pool(name="const", bufs=1))
        wpool = ctx.enter_context(tc.tile_pool(name="wpool", bufs=1))
        qkv = ctx.enter_context(tc.tile_pool(name="qkv", bufs=1))
        xsl = ctx.enter_context(tc.tile_pool(name="xsl", bufs=2))
        sp_ps = ctx.enter_context(tc.tile_pool(name="sp_ps", bufs=2, space="PSUM"))
        o_ps = ctx.enter_context(tc.tile_pool(name="o_ps", bufs=2, space="PSUM"))
        # dedicated ring for proj/yout/bcast PSUM so their allocations don't
        # serialize behind spair's exp readers at slab boundaries
        pj_ps = ctx.enter_context(tc.tile_pool(name="pj_ps", bufs=2, space="PSUM"))
        ptp = ctx.enter_context(tc.tile_pool(name="ptp", bufs=4))
        stg = ctx.enter_context(tc.tile_pool(name="stg", bufs=6))
        ysb = ctx.enter_context(tc.tile_pool(name="ysb", bufs=4))

        # ---- constants / warmups ----
        tri = const.tile([P, P], BF16, name="tri")
        # dummy matmul burst: ~4.5us of PE activity during the DMA-bound
        # preamble flips the HAM clock gate to 8/8 before real work arrives
        wsrc = const.tile([P, P], BF16, name="wsrc")
        nc.vector.memset(wsrc[:], 0.0)
        wps = sp_ps.tile([P, 512], F32, name="wps", tag="sp")
        for _ in range(34):
            nc.tensor.matmul(out=wps[:, 0:P], lhsT=wsrc[:], rhs=wsrc[:],
                             start=True, stop=True)

        # NOTE: a DMA-gated "adaptive warmup staircase" was tried here and
        # REGRESSED: HAM only unthrottles on ~3.4us of dense PE activity, and
        # sem-gated dummies trickle too sparsely — the whole early phase ran
        # at half clock (first K=8/8 at 19.5us instead of ~11us).
        # touch Exp early so the ACT table load happens in the idle preamble
        warm_src = const.tile([1, 1], F32, name="warm_src")
        nc.vector.memset(warm_src[:], 0.0)
        warm = const.tile([1, 1], F32, name="warm")
        nc.scalar.activation(warm[:], warm_src[:], mybir.ActivationFunctionType.Exp)
        # touch partition_broadcast early so the gpsimd library IRAM load
        # (~6us) happens in the idle preamble
        wpb = const.tile([P, 64], BF16, name="wpb")
        nc.vector.memset(wpb[0:1, :], 1.0)
        nc.gpsimd.partition_broadcast(wpb[:, :], wpb[0:1, :])
        # denominator-broadcast selectors for the K=1 PE path (base-64 rows):
        # bcast = oselB[r]^T (x) inv[r]; even heads r=64 -> out rows 0:64
        oselB = const.tile([65, 2, P], BF16, name="oselB")
        nc.vector.memset(oselB[0:1, :, :], 0.0)
        nc.vector.memset(oselB[64:65, :, :], 0.0)
        nc.vector.memset(oselB[64:65, 0, 0:64], 1.0)

        # ---- persistent tensors ----
        wq_s = wpool.tile([P, DC, F], BF16, name="wq_s")
        wk_s = wpool.tile([P, DC, F], BF16, name="wk_s")
        wv_s = wpool.tile([P, DC, F], BF16, name="wv_s")
        wo2 = wpool.tile([P, 2, D], BF16, name="wo2")
        qT = qkv.tile([P, 2, T], BF16, name="qT")        # [2 heads x dh, jb, t]
        kTz0 = qkv.tile([P, 2, T], BF16, name="kTz0")    # [k_even; 0]
        kTz1 = qkv.tile([P, 2, T], BF16, name="kTz1")    # [0; k_odd]
        vp = qkv.tile([P, TT, HG, P], BF16, name="vp")   # padded V', parity layouts
        oT = qkv.tile([P, 2, T], BF16, name="oT")        # normalized o^T [f, t]

        # ---- preamble DMAs + fills ----
        # Ring split: x slab 0 on sync (4 chunks for progressive arrival),
        # wq/wk/tri on scalar, wv/wo on gpsimd, x slab 1+ alternate
        # vector/sync.  vp (padded V' with denom ones-columns) is generated
        # on-device: data cols are overwritten by emit_v, the ones columns
        # are memset here, and the unused pad cols only feed never-read PSUM
        # partitions (they still get zeroed by the full memset below so the
        # race checker sees initialized reads).
        # x slab 0 is the critical startup transfer (~136 GB/s per ring):
        # split it sync/scalar so it lands ~2us earlier
        # single-dc chunks: the qs=0 projections chase slab 0's arrival, and
        # finer chunks keep the per-chunk stall under the HAM idle window
        xs_next = xsl.tile([P, DC, 512], BF16, name="xs")
        for c in range(DC):
            nc.sync.dma_start(out=xs_next[:, c:c + 1, :],
                              in_=xt_d[:, 0, c:c + 1, :])
        nc.scalar.dma_start(out=wq_s[:, 0:4], in_=wq_d[:, 0:4])
        nc.scalar.dma_start(out=wq_s[:, 4:8], in_=wq_d[:, 4:8])
        nc.scalar.dma_start(out=wk_s[:, 0:4], in_=wk_d[:, 0:4])
        nc.scalar.dma_start(out=wk_s[:, 4:8], in_=wk_d[:, 4:8])
        nc.scalar.dma_start(out=tri[:], in_=tri_d[:])
        # wv/wo (not needed until ~16us/~25us) are gated behind x slab 0's
        # last chunks with tiny dependency-creating writes, so the startup's
        # chip-HBM bandwidth goes to the critical slab0+wq/wk path first
        nc.gpsimd.tensor_copy(wv_s[0:1, 0, 0:1], xs_next[0:1, 6, 0:1])
        nc.gpsimd.dma_start(out=wv_s[:], in_=wv_d[:])
        nc.gpsimd.tensor_copy(wo2[0:1, 0, 0:1], xs_next[0:1, 7, 0:1])
        nc.gpsimd.dma_start(out=wo2[:], in_=wo_d[:])
        # slab-1 prefetch, also gated behind slab 0 (hoisted here so its gates
        # precede the big gpsimd memset below in that engine's queue)
        xs1_pre = xsl.tile([P, DC, 512], BF16, name="xs")
        nc.gpsimd.tensor_copy(xs1_pre[0:1, 0, 0:1], xs_next[0:1, 7, 0:1])
        nc.gpsimd.tensor_copy(xs1_pre[0:1, 4, 0:1], xs_next[0:1, 7, 0:1])
        nc.sync.dma_start(out=xs1_pre[:, 0:4], in_=xt_d[:, 1, 0:4])
        nc.scalar.dma_start(out=xs1_pre[:, 4:8], in_=xt_d[:, 1, 4:8])
        # split the big zero-fills across gpsimd/vector so the vector queue is
        # free for the first qkv evictions as soon as projections land
        nc.gpsimd.memset(kTz0[64:128, :, :], 0.0)
        nc.vector.memset(kTz1[0:64, :, :], 0.0)
        # vp init: ones columns (softmax denominators) + zero pad strips; the
        # data columns are fully written by emit_v before any AV reads them
        vpar = vp[:].rearrange("p tt (hp par) c -> p tt hp par c", par=2)
        nc.vector.memset(vpar[:, :, :, 0, DH:P], 0.0)
        nc.vector.memset(vpar[:, :, :, 1, 0:DH], 0.0)
        nc.vector.memset(vpar[:, :, :, 0, DH:DH + 1], 1.0)
        nc.vector.memset(vpar[:, :, :, 1, 0:1], 1.0)

        # ---- emission helpers ----
        def emit_slab(ts):
            nonlocal xs_next
            xs = xs_next
            if ts + 1 < QS:
                if ts == 0:
                    xs_next = xs1_pre  # prefetched in the preamble
                else:
                    xs_next = xsl.tile([P, DC, 512], BF16, name="xs")
                    nc.sync.dma_start(out=xs_next[:], in_=xt_d[:, ts + 1])
            sl = slice(ts * 512, (ts + 1) * 512)

            def emit_proj(w_s, which, jb):
                pp = pj_ps.tile([P, 512], F32, name="pp", tag="pj")
                for dc in range(DC):
                    nc.tensor.matmul(
                        out=pp[:],
                        lhsT=w_s[:, dc, jb * P:(jb + 1) * P],
                        rhs=xs[:, dc, :],
                        start=(dc == 0),
                        stop=(dc == DC - 1),
                    )
                if which == "q":
                    nc.vector.tensor_copy(qT[:, jb, sl], pp[:])
                else:
                    nc.vector.tensor_copy(kTz0[0:64, jb, sl], pp[0:64, :])
                    nc.vector.tensor_copy(kTz1[64:128, jb, sl], pp[64:128, :])

            deferred = []
            if ts == 0:
                # slab 0's projections chase the chunked x DMA; emit them
                # chunk-major with 4 open accumulation groups (q's on the pj
                # ring, k's borrowing the still-idle sp ring) so each arriving
                # chunk unblocks ~0.9us of back-to-back PE work
                pps = {}
                for which, jb in (("q", 0), ("q", 1)):
                    pps[(which, jb)] = pj_ps.tile([P, 512], F32, name="pp",
                                                  tag="pj")
                for which, jb in (("k", 0), ("k", 1)):
                    pps[(which, jb)] = sp_ps.tile([P, 512], F32, name="bcast",
                                                  tag="sp")
                for dc in range(DC):
                    for (which, jb), pp in pps.items():
                        w_s = wq_s if which == "q" else wk_s
                        nc.tensor.matmul(
                            out=pp[:],
                            lhsT=w_s[:, dc, jb * P:(jb + 1) * P],
                            rhs=xs[:, dc, :],
                            start=(dc == 0),
                            stop=(dc == DC - 1),
                        )
                for (which, jb), pp in pps.items():
                    if which == "q":
                        nc.vector.tensor_copy(qT[:, jb, sl], pp[:])
                    else:
                        nc.vector.tensor_copy(kTz0[0:64, jb, sl], pp[0:64, :])
                        nc.vector.tensor_copy(kTz1[64:128, jb, sl], pp[64:128, :])
            else:
                for jb in range(2):
                    emit_proj(wq_s, "q", jb)
                for jb in range(2):
                    deferred.append(lambda b=jb: emit_proj(wk_s, "k", b))

            def emit_v(j, tt):
                pv = pj_ps.tile([P, F], F32, name="pv", tag="pj")
                for dc in range(DC):
                    nc.tensor.matmul(
                        out=pv[:],
                        lhsT=xs[:, dc, j * P:(j + 1) * P],
                        rhs=wv_s[:, dc, :],
                        start=(dc == 0),
                        stop=(dc == DC - 1),
                    )
                pvv = pv[:].rearrange("p (hp par dh) -> p hp par dh", hp=2, par=2, dh=DH)
                ve = vp[:, tt, :, :].rearrange("p (hp par) c -> p hp par c", par=2)
                nc.vector.tensor_copy(ve[:, :, 0, 0:DH], pvv[:, :, 0, :])
                nc.vector.tensor_copy(ve[:, :, 1, DH:P], pvv[:, :, 1, :])
            return deferred + [(lambda a=j_, b=tt_: emit_v(a, b)) for j_, tt_ in enumerate(range(4 * ts, 4 * ts + 4))]

        state = {}

        def emit_S(qs, h, kp):
            jbh, par = h // 2, h % 2
            kTz = kTz0 if par == 0 else kTz1
            q0 = qs * 512
            spair = sp_ps.tile([P, 1024], F32, name="spair", tag="sp")
            for half in range(2):
                kt = 2 * kp + half
                k0 = kt * P
                sreg = spair[:, half * 512:(half + 1) * 512]
                lhsk = kTz[:, jbh, k0:k0 + P]
                rhsq = qT[:, jbh, :]
                d = k0 - q0
                if d > 0:
                    nc.tensor.matmul(out=sreg[:, d:512], lhsT=lhsk,
                                     rhs=rhsq[:, q0 + d:q0 + 512],
                                     start=True, stop=True)
                else:
                    nc.tensor.matmul(out=sreg, lhsT=lhsk,
                                     rhs=rhsq[:, q0:q0 + 512],
                                     start=True, stop=True)
            state[(qs, h, kp)] = spair

        def emit_exp(qs, h, kp):
            spair = state[(qs, h, kp)]
            q0 = qs * 512
            pt = ptp.tile([P, 1024], BF16, name="pt")
            d1 = (2 * kp + 1) * P - q0
            if d1 > 0:
                # diagonal pair: exp only the written PSUM regions
                d0 = max(d1 - P, 0)
                nc.scalar.activation(pt[:, d0:512], spair[:, d0:512],
                                     mybir.ActivationFunctionType.Exp, scale=0.125)
                nc.scalar.activation(pt[:, 512 + d1:1024], spair[:, 512 + d1:1024],
                                     mybir.ActivationFunctionType.Exp, scale=0.125)
            else:
                nc.scalar.activation(pt[:], spair[:],
                                     mybir.ActivationFunctionType.Exp, scale=0.125)
            # in-tile causal triangle on diagonal k-tiles
            for half in range(2):
                d = (2 * kp + half) * P - q0
                if d >= 0:
                    c = half * 512 + d
                    nc.vector.tensor_mul(pt[:, c:c + P], pt[:, c:c + P], tri[:])
            state[(qs, h, kp)] = (spair, pt)

        def emit_AV(qs, h, kp, nkt):
            _, pt = state.pop((qs, h, kp))
            q0 = qs * 512
            if kp == 0:
                state[(qs, h)] = o_ps.tile([P, 512], F32, name="opsum")
            opsum = state[(qs, h)]
            for half in range(2):
                kt = 2 * kp + half
                d = max(kt * P - q0, 0)
                nc.tensor.matmul(
                    out=opsum[:, d:512],
                    lhsT=vp[:, kt, h, :],
                    rhs=pt[:, half * 512 + d:half * 512 + 512],
                    start=(kt == 0),
                    stop=(kt == nkt - 1),
                )

        def emit_normA(qs, h):
            # full-tile approx reciprocal straight from PSUM (custom-DVE ops
            # only work at base partition 0); only row r is meaningful
            opsum = state[(qs, h)]
            r = DH if h % 2 == 0 else 0
            invf = stg.tile([P, 512], F32, name="invf")
            nc.vector.reciprocal_approx_fast(invf[:], opsum[:])
            invb = stg.tile([DH + 1, 512], BF16, name="invb")
            nc.vector.tensor_copy(invb[r:r + 1, :], invf[r:r + 1, :])
            state[(qs, h, "inv")] = (invb, r)

        def emit_normB(qs, h, drain=False):
            opsum = state.pop((qs, h))
            invb, r = state.pop((qs, h, "inv"))
            jbh, par = h // 2, h % 2
            q0 = qs * 512
            rows = slice(0, DH) if par == 0 else slice(DH, P)
            if par == 1:
                # denom at partition 0: gpsimd broadcast (full-width out; only
                # base-0 in / base-0 out are implemented in the Q7 ucode)
                bsb = stg.tile([P, 512], BF16, name="bsb")
                nc.gpsimd.partition_broadcast(bsb[:, :], invb[0:1, :])
                nc.vector.tensor_mul(oT[rows, jbh, q0:q0 + 512], opsum[rows, :], bsb[rows, :])
            else:
                # denom at partition 64: rank-1 PE broadcast (verified at base 64).
                # Mid-kernel it rides the sp ring (recycles a long-drained spair
                # slot; pj/o_ps would recycle buffers this normB's own consumers
                # still hold).  In the drain the sp ring holds the pre-opened
                # fc0 y-tile halves, so it rides o_ps there instead (recycling
                # the already-normed previous head's opsum).
                if drain:
                    bcast = o_ps.tile([P, 512], F32, name="opsum")
                else:
                    bcast = sp_ps.tile([P, 512], F32, name="bcast", tag="sp")
                nc.tensor.matmul(out=bcast[:], lhsT=oselB[64:65, 0, :],
                                 rhs=invb[64:65, :], start=True, stop=True)
                bsb = stg.tile([P, 512], BF16, name="bsb")
                nc.vector.tensor_copy(bsb[rows, :], bcast[rows, :])
                nc.vector.tensor_mul(oT[rows, jbh, q0:q0 + 512], opsum[rows, :], bsb[rows, :])

        def emit_ytile(qs, tt, e, drain=False):
            yt = ysb.tile([P, D], BF16, name="yt")
            for jh in range(2):
                yp = pj_ps.tile([P, 512], F32, name="pp", tag="pj")
                for fc in range(2):
                    nc.tensor.matmul(
                        out=yp[:],
                        lhsT=oT[:, fc, tt * P:(tt + 1) * P],
                        rhs=wo2[:, fc, jh * 512:(jh + 1) * 512],
                        start=(fc == 0),
                        stop=(fc == 1),
                    )
                nc.vector.tensor_copy(yt[:, jh * 512:(jh + 1) * 512], yp[:])
                # jh1 dispatch rides gpsimd (SWDGE): the scalar queue is the
                # exp engine and its ~600ns dispatches were adding to the
                # slab-boundary activation backlog
                eng = nc.sync if jh == 0 else nc.gpsimd
                eng.dma_start(out=y_d[tt * P:(tt + 1) * P, jh * 512:(jh + 1) * 512],
                              in_=yt[:, jh * 512:(jh + 1) * 512])

        # ---- fused pipeline ----
        # qs=3 head order [1,0,3,2]: the last-processed head is even-parity,
        # so the final normB on the drain path is the cheap PE-broadcast, and
        # both fc0 heads (0,1) finish early enough to overlap the drain.
        head_order = {qs: list(range(HG)) for qs in range(QS)}
        head_order[QS - 1] = [1, 0, 3, 2]
        last_head = {qs: head_order[qs][-1] for qs in range(QS)}
        steps = []
        first_step_of_qs = {}
        for qs in range(QS):
            first_step_of_qs[len(steps)] = qs
            for h in head_order[qs]:
                nkt = 4 * qs + 4
                for kp in range(nkt // 2):
                    steps.append((qs, h, kp, nkt))

        todo = []

        def flush(i):
            while todo and todo[0][0] <= i:
                todo.pop(0)[1]()

        nsteps = len(steps)
        for i in range(nsteps):
            if i in first_step_of_qs:
                for vj, fn_ in enumerate(emit_slab(first_step_of_qs[i])):
                    todo.append((i + vj, fn_))
                todo.sort(key=lambda e: e[0])
            qs, h, kp, nkt = steps[i]
            emit_S(qs, h, kp)
            flush(i)
            if i >= 1:
                pqs, ph_, pkp, _ = steps[i - 1]
                emit_exp(pqs, ph_, pkp)
            if i >= 2:
                pqs, ph_, pkp, pnkt = steps[i - 2]
                emit_AV(pqs, ph_, pkp, pnkt)
                if pkp == pnkt // 2 - 1:
                    emit_normA(pqs, ph_)
                    todo.append((i + min(5, 2 * (pqs + 1) + 1), lambda q=pqs, hh=ph_: emit_normB(q, hh)))
                    if ph_ == last_head[pqs]:
                        for j, tt in enumerate(range(4 * pqs, 4 * pqs + 4)):
                            todo.append((i + 6 + j,
                                         lambda q=pqs, t_=tt, e=j: emit_ytile(q, t_, e)))
        # drain: last exp/AV, then overlap the final norm chain (even-parity
        # head -> PE broadcast path) with the fc0 halves of the first two
        # output tiles.  Only 2 yp tiles are pre-opened: the 3rd sp_ps buffer
        # must stay free for the normB bcast matmul (a 3rd yp would recycle
        # its buffer and deadlock against the un-evicted fc1 halves).
        emit_exp(*steps[nsteps - 1][:3])
        last_norm = None
        for i in (nsteps - 2, nsteps - 1):
            qs, h, kp, nkt = steps[i]
            emit_AV(qs, h, kp, nkt)
            if kp == nkt // 2 - 1:
                emit_normA(qs, h)
                last_norm = (qs, h)
        for _, fn in todo:
            fn()
        lqs = QS - 1

        # pre-open y tiles 12 (sp ring — spair traffic is over) and 13 (pj
        # ring) and run their fc0 matmuls so the PE stays busy through the
        # final normB chain; the drain normB's bcast rides o_ps instead
        tt0 = 4 * lqs
        pre = []
        with tc.high_priority(offset=300):
            for tt, pool, tag in ((tt0, sp_ps, "sp"), (tt0 + 1, pj_ps, "pj")):
                yps = []
                for jh in range(2):
                    yp = pool.tile([P, 512], F32,
                                   name="bcast" if tag == "sp" else "pp", tag=tag)
                    nc.tensor.matmul(out=yp[:], lhsT=oT[:, 0, tt * P:(tt + 1) * P],
                                     rhs=wo2[:, 0, jh * 512:(jh + 1) * 512],
                                     start=True, stop=False)
                    yps.append(yp)
                pre.append((tt, yps))
        emit_normB(*last_norm, drain=True)
        # drain evictions split DVE / scalar-Copy (the loaded exp table set
        # also serves Copy, so no ACT_TABLE_LOAD) to halve the serial tail
        for tt, yps in pre:
            yt = ysb.tile([P, D], BF16, name="yt")
            for jh in range(2):
                nc.tensor.matmul(out=yps[jh][:],
                                 lhsT=oT[:, 1, tt * P:(tt + 1) * P],
                                 rhs=wo2[:, 1, jh * 512:(jh + 1) * 512],
                                 start=False, stop=True)
                if jh == 0:
                    nc.vector.tensor_copy(yt[:, 0:512], yps[0][:])
                    nc.sync.dma_start(out=y_d[tt * P:(tt + 1) * P, 0:512],
                                      in_=yt[:, 0:512])
                else:
                    nc.scalar.activation(yt[:, 512:D], yps[1][:],
                                         mybir.ActivationFunctionType.Copy)
                    nc.scalar.dma_start(out=y_d[tt * P:(tt + 1) * P, 512:D],
                                        in_=yt[:, 512:D])
        # last two tiles ride the (now idle) o_ps pool so they don't wait for
        # the pj ring to recycle
        for tt in range(tt0 + 2, tt0 + 4):
            yph = [o_ps.tile([P, 512], F32, name="opsum") for _ in range(2)]
            for fc in range(2):
                for jh in range(2):
                    nc.tensor.matmul(
                        out=yph[jh][:],
                        lhsT=oT[:, fc, tt * P:(tt + 1) * P],
                        rhs=wo2[:, fc, jh * 512:(jh + 1) * 512],
                        start=(fc == 0),
                        stop=(fc == 1),
                    )
            yt2 = ysb.tile([P, D], BF16, name="yt")
            nc.vector.tensor_copy(yt2[:, 0:512], yph[0][:])
            nc.sync.dma_start(out=y_d[tt * P:(tt + 1) * P, 0:512],
                              in_=yt2[:, 0:512])
            nc.scalar.activation(yt2[:, 512:D], yph[1][:],
                                 mybir.ActivationFunctionType.Copy)
            nc.scalar.dma_start(out=y_d[tt * P:(tt + 1) * P, 512:D],
                                in_=yt2[:, 512:D])

    nc.compile()
    return nc


def make_core_inputs(x, Wq, Wk, Wv, Wo):
    import ml_dtypes
    bf = ml_dtypes.bfloat16

    tri = np.triu(np.ones((P, P), dtype=np.float32)).astype(bf)

    def pmajor(w, chunks):  # [chunks*P, f] -> [P, chunks, f]
        return np.ascontiguousarray(
            w.reshape(chunks, P, -1).transpose(1, 0, 2)).astype(bf)

    in_maps = []
    for c in range(NCORES):
        b, hg = c // HG, c % HG
        s = slice(hg * F, (hg + 1) * F)
        # x^T slab-major: [D, T] -> [P, QS, DC, 512]
        xt = np.ascontiguousarray(
            x[b].T.reshape(DC, P, QS, 512).transpose(1, 2, 0, 3)).astype(bf)
        in_maps.append({
            "xt": xt,
            "wq": pmajor(Wq[:, s], DC),
            "wk": pmajor(Wk[:, s], DC),
            "wv": pmajor(Wv[:, s], DC),
            "wo": pmajor(Wo[s, :], 2),
            "tri": tri,
        })
    return in_maps


_NC_CACHE = None


def _get_nc():
    global _NC_CACHE
    if _NC_CACHE is None:
        _NC_CACHE = build_nc()
    return _NC_CACHE


def kernel(x, Wq, Wk, Wv, Wo):
    global LAST_RESULTS
    _install_ntff_hook()
    from concourse.bass_utils import run_bass_kernel_spmd

    x = np.asarray(x, dtype=np.float32)
    Wq = np.asarray(Wq, dtype=np.float32)
    Wk = np.asarray(Wk, dtype=np.float32)
    Wv = np.asarray(Wv, dtype=np.float32)
    Wo = np.asarray(Wo, dtype=np.float32)

    nc = _get_nc()
    in_maps = make_core_inputs(x, Wq, Wk, Wv, Wo)
    res = run_bass_kernel_spmd(nc, in_maps, list(range(NCORES)))
    LAST_RESULTS = res

    out = np.zeros((B, T, D), dtype=np.float32)
    for c in range(NCORES):
        out[c // HG] += np.asarray(res.results[c]["y"], dtype=np.float32)
    return out



# revision 60
# speedup vs baseline: 1.0060x; 1.0060x over previous
"""Causal self-attention (B=2, T=2048, D=1024, H=16) on 8 TRN2 NeuronCores.

Sharding: data-parallel over batch (2) x tensor-parallel over head groups (4).
Each core handles 1 batch x 4 heads: Wq/Wk/Wv column-sharded, Wo row-sharded;
each core emits a partial (T, D) output and the host sums 4 partials per batch.

v2 design (vs the fp32r baseline):
  - x is transposed on the HOST and shipped as bf16 [P, QS, DC, 512]
    slab-major, eliminating all 128 PE transposes and their PSUM evictions.
  - All matmul operands are bf16 (fp32 PSUM accumulation): enables the PE's
    fast-weight-load path (fp32r blocks FWL), halves eviction/DMA bytes.
  - No mask-inject matmuls: diagonal S tiles compute only columns [d, 512);
    exp is restricted to the written PSUM region (split calls on diagonal
    k-pairs), and the in-tile causal triangle is applied post-exp as a bf16
    tensor_mul with a host-provided [128,128] upper-triangular mask.
  - All PSUM evictions run on DVE; the scalar engine does (almost) only exp.
  - Softmax denominators ride in V' ones-columns (parity layout: even heads
    col 64 / rows 0-63 data, odd heads col 0 / rows 64-127 data); per (qs,h)
    the denom row is folded via SBUF DMA, reciprocal'd on DVE, unfolded and
    rank-1-broadcast on PE, then fused into the oT eviction multiply.

v3 changes (163.0us -> 152.4us), all pipeline/startup/drain plumbing:
  - V' padding is generated on-device (pad-strip + ones-column memsets)
    instead of DMAing a 2MB host constant; the big zero-fills are split
    gpsimd/vector so the vector queue is free for the first qkv evictions
    (the DVE preamble pile-up was gating the whole qs=0 phase).
  - Startup DMA rings rebalanced: x slab 0 chunked on sync, wq/wk/tri on
    scalar, wv/wo on gpsimd, x slab 1 split across both HWDGE rings (it was
    landing at ~26us on the overloaded scalar ring, stalling qs=1).
  - proj/yout/bcast PSUM moved off the spair ring into a dedicated 2-bank
    ring (their allocations serialized behind spair's exp readers at every
    slab boundary); spair drops to 2 bufs.
  - y DMAs issue per 512-column half on sync+gpsimd queues mid-kernel
    (keeping ~600ns dispatches off the scalar/exp engine) and sync+scalar
    in the drain (where scalar is idle).
  - x slab 0 ships as 8 single-dc chunks; the qs=0 projections chase its
    arrival (~14us, chip-HBM-bound: all 8 cores pull startup data at once)
    and finer chunks keep each stall under the HAM idle window.
  - Drain y tiles 12/13 pre-open their fc0 matmuls on the idle spair/pj
    rings (high_priority) to span the final normB chain; the drain normB
    bcast rides o_ps; tiles 14/15 follow on o_ps.
  - qs=3 runs heads in order [1,0,3,2] so the final normB is the cheap
    PE-broadcast path, and y-tile-12's fc0 matmuls are pre-opened
    (high_priority) to keep the PE warm through the final norm chain.

Explored and rejected (measured on HW via probe kernel): 64-row-tiled S
matmuls give a true 2x when paired back-to-back (108.8 ns per 512-col MM),
but interleaving 64-row and 128-row tile modes costs ~105ns per mode switch,
erasing the win at the per-step granularity PSUM allows (8 banks cap the S
burst depth at 2). fp8 attention fails the 2e-2 gate (~4-12% output noise).
"""

import sys, os, types

sys.path.insert(0, "/opt/trn_rl_repo")

import numpy as np
from contextlib import ExitStack

import concourse.bass as bass
import concourse.mybir as mybir
import concourse.tile as tile
from concourse import bacc

B, T, D, H = 2, 2048, 1024, 16
DH = D // H          # 64
NCORES = 8
HG = 4               # heads per core
F = HG * DH          # 256 local features per core
P = 128
F32 = mybir.dt.float32
F32R = mybir.dt.float32r
BF16 = mybir.dt.bfloat16
FP8 = mybir.dt.float8e4

TT = T // P          # 16 t-tiles
QS = T // 512        # 4 q-slabs
DC = D // P          # 8 d-chunks

LAST_RESULTS = None  # BassKernelResults of the most recent hardware run


def _install_ntff_hook():
    if "antenv.axon_hooks" in sys.modules:
        return
    try:
        import antenv
        from trn_agent_boot.trn_boot import _ntff_profile_via_ctypes

        m = types.ModuleType("antenv.axon_hooks")
        h = _ntff_profile_via_ctypes("/opt/axon/libaxon_pjrt.so")
        m.get_axon_ntff_profile_hook = lambda: h
        m.set_axon_ntff_profile_hook = lambda hh: None
        sys.modules["antenv.axon_hooks"] = m
        antenv.axon_hooks = m
    except Exception:
        pass


def build_nc():
    nc = bacc.Bacc("TRN2", target_bir_lowering=False, debug=False)

    # x^T slab-major: [P, QS, DC, 512] so one q-slab is a contiguous
    # 8KB-per-partition DMA run
    xt_d = nc.dram_tensor("xt", [P, QS, DC, 512], BF16, kind="ExternalInput").ap()
    wq_d = nc.dram_tensor("wq", [P, DC, F], BF16, kind="ExternalInput").ap()
    wk_d = nc.dram_tensor("wk", [P, DC, F], BF16, kind="ExternalInput").ap()
    wv_d = nc.dram_tensor("wv", [P, DC, F], BF16, kind="ExternalInput").ap()
    wo_d = nc.dram_tensor("wo", [P, 2, D], BF16, kind="ExternalInput").ap()
    tri_d = nc.dram_tensor("tri", [P, P], BF16, kind="ExternalInput").ap()
    y_d = nc.dram_tensor("y", [T, D], BF16, kind="ExternalOutput").ap()

    with tile.TileContext(nc) as tc, ExitStack() as ctx:
        const = ctx.enter_context(tc.tile_pool(name="const", bufs=1))
        wpool = ctx.enter_context(tc.tile_pool(name="wpool", bufs=1))
        qkv = ctx.enter_context(tc.tile_pool(name="qkv", bufs=1))
        xsl = ctx.enter_context(tc.tile_pool(name="xsl", bufs=2))
        sp_ps = ctx.enter_context(tc.tile_pool(name="sp_ps", bufs=2, space="PSUM"))
        o_ps = ctx.enter_context(tc.tile_pool(name="o_ps", bufs=2, space="PSUM"))
        # dedicated ring for proj/yout/bcast PSUM so their allocations don't
        # serialize behind spair's exp readers at slab boundaries
        pj_ps = ctx.enter_context(tc.tile_pool(name="pj_ps", bufs=2, space="PSUM"))
        ptp = ctx.enter_context(tc.tile_pool(name="ptp", bufs=4))
        stg = ctx.enter_context(tc.tile_pool(name="stg", bufs=6))
        ysb = ctx.enter_context(tc.tile_pool(name="ysb", bufs=4))

        # ---- constants / warmups ----
        tri = const.tile([P, P], BF16, name="tri")
        # dummy matmul burst: ~4.5us of PE activity during the DMA-bound
        # preamble flips the HAM clock gate to 8/8 before real work arrives
        wsrc = const.tile([P, P], BF16, name="wsrc")
        nc.vector.memset(wsrc[:], 0.0)
        wps = sp_ps.tile([P, 512], F32, name="wps", tag="sp")
        for _ in range(34):
            nc.tensor.matmul(out=wps[:, 0:P], lhsT=wsrc[:], rhs=wsrc[:],
                             start=True, stop=True)

        # NOTE: a DMA-gated "adaptive warmup staircase" was tried here and
        # REGRESSED: HAM only unthrottles on ~3.4us of dense PE activity, and
        # sem-gated dummies trickle too sparsely — the whole early phase ran
        # at half clock (first K=8/8 at 19.5us instead of ~11us).
        # touch Exp early so the ACT table load happens in the idle preamble
        warm_src = const.tile([1, 1], F32, name="warm_src")
        nc.vector.memset(warm_src[:], 0.0)
        warm = const.tile([1, 1], F32, name="warm")
        nc.scalar.activation(warm[:], warm_src[:], mybir.ActivationFunctionType.Exp)
        # touch partition_broadcast early so the gpsimd library IRAM load
        # (~6us) happens in the idle preamble
        wpb = const.tile([P, 64], BF16, name="wpb")
        nc.vector.memset(wpb[0:1, :], 1.0)
        nc.gpsimd.partition_broadcast(wpb[:, :], wpb[0:1, :])
        # denominator-broadcast selectors for the K=1 PE path (base-64 rows):
        # bcast = oselB[r]^T (x) inv[r]; even heads r=64 -> out rows 0:64
        oselB = const.tile([65, 2, P], BF16, name="oselB")
        nc.vector.memset(oselB[0:1, :, :], 0.0)
        nc.vector.memset(oselB[64:65, :, :], 0.0)
        nc.vector.memset(oselB[64:65, 0, 0:64], 1.0)

        # ---- persistent tensors ----
        wq_s = wpool.tile([P, DC, F], BF16, name="wq_s")
        wk_s = wpool.tile([P, DC, F], BF16, name="wk_s")
        wv_s = wpool.tile([P, DC, F], BF16, name="wv_s")
        wo2 = wpool.tile([P, 2, D], BF16, name="wo2")
        qT = qkv.tile([P, 2, T], BF16, name="qT")        # [2 heads x dh, jb, t]
        kTz0 = qkv.tile([P, 2, T], BF16, name="kTz0")    # [k_even; 0]
        kTz1 = qkv.tile([P, 2, T], BF16, name="kTz1")    # [0; k_odd]
        vp = qkv.tile([P, TT, HG, P], BF16, name="vp")   # padded V', parity layouts
        oT = qkv.tile([P, 2, T], BF16, name="oT")        # normalized o^T [f, t]

        # ---- preamble DMAs + fills ----
        # Ring split: x slab 0 on sync (4 chunks for progressive arrival),
        # wq/wk/tri on scalar, wv/wo on gpsimd, x slab 1+ alternate
        # vector/sync.  vp (padded V' with denom ones-columns) is generated
        # on-device: data cols are overwritten by emit_v, the ones columns
        # are memset here, and the unused pad cols only feed never-read PSUM
        # partitions (they still get zeroed by the full memset below so the
        # race checker sees initialized reads).
        # x slab 0 is the critical startup transfer (~136 GB/s per ring):
        # split it sync/scalar so it lands ~2us earlier
        # single-dc chunks: the qs=0 projections chase slab 0's arrival, and
        # finer chunks keep the per-chunk stall under the HAM idle window
        xs_next = xsl.tile([P, DC, 512], BF16, name="xs")
        for c in range(DC):
            nc.sync.dma_start(out=xs_next[:, c:c + 1, :],
                              in_=xt_d[:, 0, c:c + 1, :])
        nc.scalar.dma_start(out=wq_s[:, 0:4], in_=wq_d[:, 0:4])
        nc.scalar.dma_start(out=wq_s[:, 4:8], in_=wq_d[:, 4:8])
        nc.scalar.dma_start(out=wk_s[:, 0:4], in_=wk_d[:, 0:4])
        nc.scalar.dma_start(out=wk_s[:, 4:8], in_=wk_d[:, 4:8])
        nc.scalar.dma_start(out=tri[:], in_=tri_d[:])
        # wv/wo (not needed until ~16us/~25us) are gated behind x slab 0's
        # last chunks with tiny dependency-creating writes, so the startup's
        # chip-HBM bandwidth goes to the critical slab0+wq/wk path first
        nc.gpsimd.tensor_copy(wv_s[0:1, 0, 0:1], xs_next[0:1, 6, 0:1])
        nc.gpsimd.dma_start(out=wv_s[:], in_=wv_d[:])
        nc.gpsimd.tensor_copy(wo2[0:1, 0, 0:1], xs_next[0:1, 7, 0:1])
        nc.gpsimd.dma_start(out=wo2[:], in_=wo_d[:])

        # split the big zero-fills across gpsimd/vector so the vector queue is
        # free for the first qkv evictions as soon as projections land
        nc.gpsimd.memset(kTz0[64:128, :, :], 0.0)
        nc.vector.memset(kTz1[0:64, :, :], 0.0)
        # vp init: ones columns (softmax denominators) + zero pad strips; the
        # data columns are fully written by emit_v before any AV reads them
        vpar = vp[:].rearrange("p tt (hp par) c -> p tt hp par c", par=2)
        nc.vector.memset(vpar[:, :, :, 0, DH:P], 0.0)
        nc.vector.memset(vpar[:, :, :, 1, 0:DH], 0.0)
        nc.vector.memset(vpar[:, :, :, 0, DH:DH + 1], 1.0)
        nc.vector.memset(vpar[:, :, :, 1, 0:1], 1.0)

        # ---- emission helpers ----
        def emit_slab(ts):
            nonlocal xs_next
            xs = xs_next
            if ts + 1 < QS:
                xs_next = xsl.tile([P, DC, 512], BF16, name="xs")
                if ts == 0:
                    # slab 1 split across both HWDGE rings, gated behind
                    # slab 0's completion (same HBM-contention reasoning)
                    nc.gpsimd.tensor_copy(xs_next[0:1, 0, 0:1], xs[0:1, 7, 0:1])
                    nc.gpsimd.tensor_copy(xs_next[0:1, 4, 0:1], xs[0:1, 7, 0:1])
                    nc.sync.dma_start(out=xs_next[:, 0:4], in_=xt_d[:, 1, 0:4])
                    nc.scalar.dma_start(out=xs_next[:, 4:8], in_=xt_d[:, 1, 4:8])
                else:
                    nc.sync.dma_start(out=xs_next[:], in_=xt_d[:, ts + 1])
            sl = slice(ts * 512, (ts + 1) * 512)

            def emit_proj(w_s, which, jb):
                pp = pj_ps.tile([P, 512], F32, name="pp", tag="pj")
                for dc in range(DC):
                    nc.tensor.matmul(
                        out=pp[:],
                        lhsT=w_s[:, dc, jb * P:(jb + 1) * P],
                        rhs=xs[:, dc, :],
                        start=(dc == 0),
                        stop=(dc == DC - 1),
                    )
                if which == "q":
                    nc.vector.tensor_copy(qT[:, jb, sl], pp[:])
                else:
                    nc.vector.tensor_copy(kTz0[0:64, jb, sl], pp[0:64, :])
                    nc.vector.tensor_copy(kTz1[64:128, jb, sl], pp[64:128, :])

            for jb in range(2):
                emit_proj(wq_s, "q", jb)
            deferred = []
            if ts == 0:
                for jb in range(2):
                    emit_proj(wk_s, "k", jb)
            else:
                for jb in range(2):
                    deferred.append(lambda b=jb: emit_proj(wk_s, "k", b))

            def emit_v(j, tt):
                pv = pj_ps.tile([P, F], F32, name="pv", tag="pj")
                for dc in range(DC):
                    nc.tensor.matmul(
                        out=pv[:],
                        lhsT=xs[:, dc, j * P:(j + 1) * P],
                        rhs=wv_s[:, dc, :],
                        start=(dc == 0),
                        stop=(dc == DC - 1),
                    )
                pvv = pv[:].rearrange("p (hp par dh) -> p hp par dh", hp=2, par=2, dh=DH)
                ve = vp[:, tt, :, :].rearrange("p (hp par) c -> p hp par c", par=2)
                nc.vector.tensor_copy(ve[:, :, 0, 0:DH], pvv[:, :, 0, :])
                nc.vector.tensor_copy(ve[:, :, 1, DH:P], pvv[:, :, 1, :])
            return deferred + [(lambda a=j_, b=tt_: emit_v(a, b)) for j_, tt_ in enumerate(range(4 * ts, 4 * ts + 4))]

        state = {}

        def emit_S(qs, h, kp):
            jbh, par = h // 2, h % 2
            kTz = kTz0 if par == 0 else kTz1
            q0 = qs * 512
            spair = sp_ps.tile([P, 1024], F32, name="spair", tag="sp")
            for half in range(2):
                kt = 2 * kp + half
                k0 = kt * P
                sreg = spair[:, half * 512:(half + 1) * 512]
                lhsk = kTz[:, jbh, k0:k0 + P]
                rhsq = qT[:, jbh, :]
                d = k0 - q0
                if d > 0:
                    nc.tensor.matmul(out=sreg[:, d:512], lhsT=lhsk,
                                     rhs=rhsq[:, q0 + d:q0 + 512],
                                     start=True, stop=True)
                else:
                    nc.tensor.matmul(out=sreg, lhsT=lhsk,
                                     rhs=rhsq[:, q0:q0 + 512],
                                     start=True, stop=True)
            state[(qs, h, kp)] = spair

        def emit_exp(qs, h, kp):
            spair = state[(qs, h, kp)]
            q0 = qs * 512
            pt = ptp.tile([P, 1024], BF16, name="pt")
            d1 = (2 * kp + 1) * P - q0
            if d1 > 0:
                # diagonal pair: exp only the written PSUM regions
                d0 = max(d1 - P, 0)
                nc.scalar.activation(pt[:, d0:512], spair[:, d0:512],
                                     mybir.ActivationFunctionType.Exp, scale=0.125)
                nc.scalar.activation(pt[:, 512 + d1:1024], spair[:, 512 + d1:1024],
                                     mybir.ActivationFunctionType.Exp, scale=0.125)
            else:
                nc.scalar.activation(pt[:], spair[:],
                                     mybir.ActivationFunctionType.Exp, scale=0.125)
            # in-tile causal triangle on diagonal k-tiles
            for half in range(2):
                d = (2 * kp + half) * P - q0
                if d >= 0:
                    c = half * 512 + d
                    nc.vector.tensor_mul(pt[:, c:c + P], pt[:, c:c + P], tri[:])
            state[(qs, h, kp)] = (spair, pt)

        def emit_AV(qs, h, kp, nkt):
            _, pt = state.pop((qs, h, kp))
            q0 = qs * 512
            if kp == 0:
                state[(qs, h)] = o_ps.tile([P, 512], F32, name="opsum")
            opsum = state[(qs, h)]
            for half in range(2):
                kt = 2 * kp + half
                d = max(kt * P - q0, 0)
                nc.tensor.matmul(
                    out=opsum[:, d:512],
                    lhsT=vp[:, kt, h, :],
                    rhs=pt[:, half * 512 + d:half * 512 + 512],
                    start=(kt == 0),
                    stop=(kt == nkt - 1),
                )

        def emit_normA(qs, h):
            # full-tile approx reciprocal straight from PSUM (custom-DVE ops
            # only work at base partition 0); only row r is meaningful
            opsum = state[(qs, h)]
            r = DH if h % 2 == 0 else 0
            invf = stg.tile([P, 512], F32, name="invf")
            nc.vector.reciprocal_approx_fast(invf[:], opsum[:])
            invb = stg.tile([DH + 1, 512], BF16, name="invb")
            nc.vector.tensor_copy(invb[r:r + 1, :], invf[r:r + 1, :])
            state[(qs, h, "inv")] = (invb, r)

        def emit_normB(qs, h, drain=False):
            opsum = state.pop((qs, h))
            invb, r = state.pop((qs, h, "inv"))
            jbh, par = h // 2, h % 2
            q0 = qs * 512
            rows = slice(0, DH) if par == 0 else slice(DH, P)
            if par == 1:
                # denom at partition 0: gpsimd broadcast (full-width out; only
                # base-0 in / base-0 out are implemented in the Q7 ucode)
                bsb = stg.tile([P, 512], BF16, name="bsb")
                nc.gpsimd.partition_broadcast(bsb[:, :], invb[0:1, :])
                nc.vector.tensor_mul(oT[rows, jbh, q0:q0 + 512], opsum[rows, :], bsb[rows, :])
            else:
                # denom at partition 64: rank-1 PE broadcast (verified at base 64).
                # Mid-kernel it rides the sp ring (recycles a long-drained spair
                # slot; pj/o_ps would recycle buffers this normB's own consumers
                # still hold).  In the drain the sp ring holds the pre-opened
                # fc0 y-tile halves, so it rides o_ps there instead (recycling
                # the already-normed previous head's opsum).
                if drain:
                    bcast = o_ps.tile([P, 512], F32, name="opsum")
                else:
                    bcast = sp_ps.tile([P, 512], F32, name="bcast", tag="sp")
                nc.tensor.matmul(out=bcast[:], lhsT=oselB[64:65, 0, :],
                                 rhs=invb[64:65, :], start=True, stop=True)
                bsb = stg.tile([P, 512], BF16, name="bsb")
                nc.vector.tensor_copy(bsb[rows, :], bcast[rows, :])
                nc.vector.tensor_mul(oT[rows, jbh, q0:q0 + 512], opsum[rows, :], bsb[rows, :])

        def emit_ytile(qs, tt, e, drain=False):
            yt = ysb.tile([P, D], BF16, name="yt")
            for jh in range(2):
                yp = pj_ps.tile([P, 512], F32, name="pp", tag="pj")
                for fc in range(2):
                    nc.tensor.matmul(
                        out=yp[:],
                        lhsT=oT[:, fc, tt * P:(tt + 1) * P],
                        rhs=wo2[:, fc, jh * 512:(jh + 1) * 512],
                        start=(fc == 0),
                        stop=(fc == 1),
                    )
                nc.vector.tensor_copy(yt[:, jh * 512:(jh + 1) * 512], yp[:])
                # jh1 dispatch rides gpsimd (SWDGE): the scalar queue is the
                # exp engine and its ~600ns dispatches were adding to the
                # slab-boundary activation backlog
                eng = nc.sync if jh == 0 else nc.gpsimd
                eng.dma_start(out=y_d[tt * P:(tt + 1) * P, jh * 512:(jh + 1) * 512],
                              in_=yt[:, jh * 512:(jh + 1) * 512])

        # ---- fused pipeline ----
        # qs=3 head order [1,0,3,2]: the last-processed head is even-parity,
        # so the final normB on the drain path is the cheap PE-broadcast, and
        # both fc0 heads (0,1) finish early enough to overlap the drain.
        head_order = {qs: list(range(HG)) for qs in range(QS)}
        head_order[QS - 1] = [1, 0, 3, 2]
        last_head = {qs: head_order[qs][-1] for qs in range(QS)}
        steps = []
        first_step_of_qs = {}
        for qs in range(QS):
            first_step_of_qs[len(steps)] = qs
            for h in head_order[qs]:
                nkt = 4 * qs + 4
                for kp in range(nkt // 2):
                    steps.append((qs, h, kp, nkt))

        todo = []

        def flush(i):
            while todo and todo[0][0] <= i:
                todo.pop(0)[1]()

        nsteps = len(steps)
        for i in range(nsteps):
            if i in first_step_of_qs:
                for vj, fn_ in enumerate(emit_slab(first_step_of_qs[i])):
                    todo.append((i + vj, fn_))
                todo.sort(key=lambda e: e[0])
            qs, h, kp, nkt = steps[i]
            emit_S(qs, h, kp)
            flush(i)
            if i >= 1:
                pqs, ph_, pkp, _ = steps[i - 1]
                emit_exp(pqs, ph_, pkp)
            if i >= 2:
                pqs, ph_, pkp, pnkt = steps[i - 2]
                emit_AV(pqs, ph_, pkp, pnkt)
                if pkp == pnkt // 2 - 1:
                    emit_normA(pqs, ph_)
                    todo.append((i + min(5, 2 * (pqs + 1) + 1), lambda q=pqs, hh=ph_: emit_normB(q, hh)))
                    if ph_ == last_head[pqs]:
                        for j, tt in enumerate(range(4 * pqs, 4 * pqs + 4)):
                            todo.append((i + 6 + j,
                                         lambda q=pqs, t_=tt, e=j: emit_ytile(q, t_, e)))
        # drain: last exp/AV, then overlap the final norm chain (even-parity
        # head -> PE broadcast path) with the fc0 halves of the first two
        # output tiles.  Only 2 yp tiles are pre-opened: the 3rd sp_ps buffer
        # must stay free for the normB bcast matmul (a 3rd yp would recycle
        # its buffer and deadlock against the un-evicted fc1 halves).
        emit_exp(*steps[nsteps - 1][:3])
        last_norm = None
        for i in (nsteps - 2, nsteps - 1):
            qs, h, kp, nkt = steps[i]
            emit_AV(qs, h, kp, nkt)
            if kp == nkt // 2 - 1:
                emit_normA(qs, h)
                last_norm = (qs, h)
        for _, fn in todo:
            fn()
        lqs = QS - 1

        # pre-open y tiles 12 (sp ring — spair traffic is over) and 13 (pj
        # ring) and run their fc0 matmuls so the PE stays busy through the
        # final normB chain; the drain normB's bcast rides o_ps instead
        tt0 = 4 * lqs
        pre = []
        with tc.high_priority(offset=300):
            for tt, pool, tag in ((tt0, sp_ps, "sp"), (tt0 + 1, pj_ps, "pj")):
                yps = []
                for jh in range(2):
                    yp = pool.tile([P, 512], F32,
                                   name="bcast" if tag == "sp" else "pp", tag=tag)
                    nc.tensor.matmul(out=yp[:], lhsT=oT[:, 0, tt * P:(tt + 1) * P],
                                     rhs=wo2[:, 0, jh * 512:(jh + 1) * 512],
                                     start=True, stop=False)
                    yps.append(yp)
                pre.append((tt, yps))
        emit_normB(*last_norm, drain=True)
        # drain evictions split DVE / scalar-Copy (the loaded exp table set
        # also serves Copy, so no ACT_TABLE_LOAD) to halve the serial tail
        for tt, yps in pre:
            yt = ysb.tile([P, D], BF16, name="yt")
            for jh in range(2):
                nc.tensor.matmul(out=yps[jh][:],
                                 lhsT=oT[:, 1, tt * P:(tt + 1) * P],
                                 rhs=wo2[:, 1, jh * 512:(jh + 1) * 512],
                                 start=False, stop=True)
                if jh == 0:
                    nc.vector.tensor_copy(yt[:, 0:512], yps[0][:])
                    nc.sync.dma_start(out=y_d[tt * P:(tt + 1) * P, 0:512],
                                      in_=yt[:, 0:512])
                else:
                    nc.scalar.activation(yt[:, 512:D], yps[1][:],
                                         mybir.ActivationFunctionType.Copy)
                    nc.scalar.dma_start(out=y_d[tt * P:(tt + 1) * P, 512:D],
                                        in_=yt[:, 512:D])
        # last two tiles ride the (now idle) o_ps pool so they don't wait for
        # the pj ring to recycle
        for tt in range(tt0 + 2, tt0 + 4):
            yph = [o_ps.tile([P, 512], F32, name="opsum") for _ in range(2)]
            for fc in range(2):
                for jh in range(2):
                    nc.tensor.matmul(
                        out=yph[jh][:],
                        lhsT=oT[:, fc, tt * P:(tt + 1) * P],
                        rhs=wo2[:, fc, jh * 512:(jh + 1) * 512],
                        start=(fc == 0),
                        stop=(fc == 1),
                    )
            yt2 = ysb.tile([P, D], BF16, name="yt")
            nc.vector.tensor_copy(yt2[:, 0:512], yph[0][:])
            nc.sync.dma_start(out=y_d[tt * P:(tt + 1) * P, 0:512],
                              in_=yt2[:, 0:512])
            nc.scalar.activation(yt2[:, 512:D], yph[1][:],
                                 mybir.ActivationFunctionType.Copy)
            nc.scalar.dma_start(out=y_d[tt * P:(tt + 1) * P, 512:D],
                                in_=yt2[:, 512:D])

    nc.compile()
    return nc


def make_core_inputs(x, Wq, Wk, Wv, Wo):
    import ml_dtypes
    bf = ml_dtypes.bfloat16

    tri = np.triu(np.ones((P, P), dtype=np.float32)).astype(bf)

    def pmajor(w, chunks):  # [chunks*P, f] -> [P, chunks, f]
        return np.ascontiguousarray(
            w.reshape(chunks, P, -1).transpose(1, 0, 2)).astype(bf)

    in_maps = []
    for c in range(NCORES):
        b, hg = c // HG, c % HG
        s = slice(hg * F, (hg + 1) * F)
        # x^T slab-major: [D, T] -> [P, QS, DC, 512]
        xt = np.ascontiguousarray(
            x[b].T.reshape(DC, P, QS, 512).transpose(1, 2, 0, 3)).astype(bf)
        in_maps.append({
            "xt": xt,
            "wq": pmajor(Wq[:, s], DC),
            "wk": pmajor(Wk[:, s], DC),
            "wv": pmajor(Wv[:, s], DC),
            "wo": pmajor(Wo[s, :], 2),
            "tri": tri,
        })
    return in_maps


_NC_CACHE = None


def _get_nc():
    global _NC_CACHE
    if _NC_CACHE is None:
        _NC_CACHE = build_nc()
    return _NC_CACHE


def kernel(x, Wq, Wk, Wv, Wo):
    global LAST_RESULTS
    _install_ntff_hook()
    from concourse.bass_utils import run_bass_kernel_spmd

    x = np.asarray(x, dtype=np.float32)
    Wq = np.asarray(Wq, dtype=np.float32)
    Wk = np.asarray(Wk, dtype=np.float32)
    Wv = np.asarray(Wv, dtype=np.float32)
    Wo = np.asarray(Wo, dtype=np.float32)

    nc = _get_nc()
    in_maps = make_core_inputs(x, Wq, Wk, Wv, Wo)
    res = run_bass_kernel_spmd(nc, in_maps, list(range(NCORES)))
    LAST_RESULTS = res

    out = np.zeros((B, T, D), dtype=np.float32)
    for c in range(NCORES):
        out[c // HG] += np.asarray(res.results[c]["y"], dtype=np.float32)
    return out



# revision 62
# speedup vs baseline: 1.0142x; 1.0082x over previous
"""Causal self-attention (B=2, T=2048, D=1024, H=16) on 8 TRN2 NeuronCores.

Sharding: data-parallel over batch (2) x tensor-parallel over head groups (4).
Each core handles 1 batch x 4 heads: Wq/Wk/Wv column-sharded, Wo row-sharded;
each core emits a partial (T, D) output and the host sums 4 partials per batch.

v2 design (vs the fp32r baseline):
  - x is transposed on the HOST and shipped as bf16 [P, QS, DC, 512]
    slab-major, eliminating all 128 PE transposes and their PSUM evictions.
  - All matmul operands are bf16 (fp32 PSUM accumulation): enables the PE's
    fast-weight-load path (fp32r blocks FWL), halves eviction/DMA bytes.
  - No mask-inject matmuls: diagonal S tiles compute only columns [d, 512);
    exp is restricted to the written PSUM region (split calls on diagonal
    k-pairs), and the in-tile causal triangle is applied post-exp as a bf16
    tensor_mul with a host-provided [128,128] upper-triangular mask.
  - All PSUM evictions run on DVE; the scalar engine does (almost) only exp.
  - Softmax denominators ride in V' ones-columns (parity layout: even heads
    col 64 / rows 0-63 data, odd heads col 0 / rows 64-127 data); per (qs,h)
    the denom row is folded via SBUF DMA, reciprocal'd on DVE, unfolded and
    rank-1-broadcast on PE, then fused into the oT eviction multiply.

v3 changes (163.0us -> 152.4us), all pipeline/startup/drain plumbing:
  - V' padding is generated on-device (pad-strip + ones-column memsets)
    instead of DMAing a 2MB host constant; the big zero-fills are split
    gpsimd/vector so the vector queue is free for the first qkv evictions
    (the DVE preamble pile-up was gating the whole qs=0 phase).
  - Startup DMA rings rebalanced: x slab 0 chunked on sync, wq/wk/tri on
    scalar, wv/wo on gpsimd, x slab 1 split across both HWDGE rings (it was
    landing at ~26us on the overloaded scalar ring, stalling qs=1).
  - proj/yout/bcast PSUM moved off the spair ring into a dedicated 2-bank
    ring (their allocations serialized behind spair's exp readers at every
    slab boundary); spair drops to 2 bufs.
  - y DMAs issue per 512-column half on sync+gpsimd queues mid-kernel
    (keeping ~600ns dispatches off the scalar/exp engine) and sync+scalar
    in the drain (where scalar is idle).
  - x slab 0 ships as 8 single-dc chunks; the qs=0 projections chase its
    arrival (~14us, chip-HBM-bound: all 8 cores pull startup data at once)
    and finer chunks keep each stall under the HAM idle window.
  - Drain y tiles 12/13 pre-open their fc0 matmuls on the idle spair/pj
    rings (high_priority) to span the final normB chain; the drain normB
    bcast rides o_ps; tiles 14/15 follow on o_ps.
  - qs=3 runs heads in order [1,0,3,2] so the final normB is the cheap
    PE-broadcast path, and y-tile-12's fc0 matmuls are pre-opened
    (high_priority) to keep the PE warm through the final norm chain.

Explored and rejected (measured on HW via probe kernel): 64-row-tiled S
matmuls give a true 2x when paired back-to-back (108.8 ns per 512-col MM),
but interleaving 64-row and 128-row tile modes costs ~105ns per mode switch,
erasing the win at the per-step granularity PSUM allows (8 banks cap the S
burst depth at 2). fp8 attention fails the 2e-2 gate (~4-12% output noise).
"""

import sys, os, types

sys.path.insert(0, "/opt/trn_rl_repo")

import numpy as np
from contextlib import ExitStack

import concourse.bass as bass
import concourse.mybir as mybir
import concourse.tile as tile
from concourse import bacc

B, T, D, H = 2, 2048, 1024, 16
DH = D // H          # 64
NCORES = 8
HG = 4               # heads per core
F = HG * DH          # 256 local features per core
P = 128
F32 = mybir.dt.float32
F32R = mybir.dt.float32r
BF16 = mybir.dt.bfloat16
FP8 = mybir.dt.float8e4

TT = T // P          # 16 t-tiles
QS = T // 512        # 4 q-slabs
DC = D // P          # 8 d-chunks

LAST_RESULTS = None  # BassKernelResults of the most recent hardware run


def _install_ntff_hook():
    if "antenv.axon_hooks" in sys.modules:
        return
    try:
        import antenv
        from trn_agent_boot.trn_boot import _ntff_profile_via_ctypes

        m = types.ModuleType("antenv.axon_hooks")
        h = _ntff_profile_via_ctypes("/opt/axon/libaxon_pjrt.so")
        m.get_axon_ntff_profile_hook = lambda: h
        m.set_axon_ntff_profile_hook = lambda hh: None
        sys.modules["antenv.axon_hooks"] = m
        antenv.axon_hooks = m
    except Exception:
        pass


def build_nc():
    nc = bacc.Bacc("TRN2", target_bir_lowering=False, debug=False)

    # x^T slab-major: [P, QS, DC, 512] so one q-slab is a contiguous
    # 8KB-per-partition DMA run
    xt_d = nc.dram_tensor("xt", [P, QS, DC, 512], BF16, kind="ExternalInput").ap()
    wq_d = nc.dram_tensor("wq", [P, DC, F], BF16, kind="ExternalInput").ap()
    wk_d = nc.dram_tensor("wk", [P, DC, F], BF16, kind="ExternalInput").ap()
    wv_d = nc.dram_tensor("wv", [P, DC, F], BF16, kind="ExternalInput").ap()
    wo_d = nc.dram_tensor("wo", [P, 2, D], BF16, kind="ExternalInput").ap()
    tri_d = nc.dram_tensor("tri", [P, P], BF16, kind="ExternalInput").ap()
    y_d = nc.dram_tensor("y", [T, D], BF16, kind="ExternalOutput").ap()

    with tile.TileContext(nc) as tc, ExitStack() as ctx:
        const = ctx.enter_context(tc.tile_pool(name="const", bufs=1))
        wpool = ctx.enter_context(tc.tile_pool(name="wpool", bufs=1))
        qkv = ctx.enter_context(tc.tile_pool(name="qkv", bufs=1))
        xsl = ctx.enter_context(tc.tile_pool(name="xsl", bufs=2))
        sp_ps = ctx.enter_context(tc.tile_pool(name="sp_ps", bufs=2, space="PSUM"))
        o_ps = ctx.enter_context(tc.tile_pool(name="o_ps", bufs=2, space="PSUM"))
        # dedicated ring for proj/yout/bcast PSUM so their allocations don't
        # serialize behind spair's exp readers at slab boundaries
        pj_ps = ctx.enter_context(tc.tile_pool(name="pj_ps", bufs=2, space="PSUM"))
        ptp = ctx.enter_context(tc.tile_pool(name="ptp", bufs=4))
        stg = ctx.enter_context(tc.tile_pool(name="stg", bufs=6))
        ysb = ctx.enter_context(tc.tile_pool(name="ysb", bufs=4))

        # ---- constants / warmups ----
        tri = const.tile([P, P], BF16, name="tri")
        # dummy matmul burst: ~4.5us of PE activity during the DMA-bound
        # preamble flips the HAM clock gate to 8/8 before real work arrives
        wsrc = const.tile([P, P], BF16, name="wsrc")
        nc.vector.memset(wsrc[:], 0.0)
        wps = sp_ps.tile([P, 512], F32, name="wps", tag="sp")
        for _ in range(30):
            nc.tensor.matmul(out=wps[:, 0:P], lhsT=wsrc[:], rhs=wsrc[:],
                             start=True, stop=True)

        # NOTE: a DMA-gated "adaptive warmup staircase" was tried here and
        # REGRESSED: HAM only unthrottles on ~3.4us of dense PE activity, and
        # sem-gated dummies trickle too sparsely — the whole early phase ran
        # at half clock (first K=8/8 at 19.5us instead of ~11us).
        # touch Exp early so the ACT table load happens in the idle preamble
        warm_src = const.tile([1, 1], F32, name="warm_src")
        nc.vector.memset(warm_src[:], 0.0)
        warm = const.tile([1, 1], F32, name="warm")
        nc.scalar.activation(warm[:], warm_src[:], mybir.ActivationFunctionType.Exp)
        # touch partition_broadcast early so the gpsimd library IRAM load
        # (~6us) happens in the idle preamble
        wpb = const.tile([P, 64], BF16, name="wpb")
        nc.vector.memset(wpb[0:1, :], 1.0)
        nc.gpsimd.partition_broadcast(wpb[:, :], wpb[0:1, :])
        # denominator-broadcast selectors for the K=1 PE path (base-64 rows):
        # bcast = oselB[r]^T (x) inv[r]; even heads r=64 -> out rows 0:64
        oselB = const.tile([65, 2, P], BF16, name="oselB")
        nc.vector.memset(oselB[0:1, :, :], 0.0)
        nc.vector.memset(oselB[64:65, :, :], 0.0)
        nc.vector.memset(oselB[64:65, 0, 0:64], 1.0)

        # ---- persistent tensors ----
        wq_s = wpool.tile([P, DC, F], BF16, name="wq_s")
        wk_s = wpool.tile([P, DC, F], BF16, name="wk_s")
        wv_s = wpool.tile([P, DC, F], BF16, name="wv_s")
        wo2 = wpool.tile([P, 2, D], BF16, name="wo2")
        qT = qkv.tile([P, 2, T], BF16, name="qT")        # [2 heads x dh, jb, t]
        kTz0 = qkv.tile([P, 2, T], BF16, name="kTz0")    # [k_even; 0]
        kTz1 = qkv.tile([P, 2, T], BF16, name="kTz1")    # [0; k_odd]
        vp = qkv.tile([P, TT, HG, P], BF16, name="vp")   # padded V', parity layouts
        oT = qkv.tile([P, 2, T], BF16, name="oT")        # normalized o^T [f, t]

        # ---- preamble DMAs + fills ----
        # Ring split: x slab 0 on sync (4 chunks for progressive arrival),
        # wq/wk/tri on scalar, wv/wo on gpsimd, x slab 1+ alternate
        # vector/sync.  vp (padded V' with denom ones-columns) is generated
        # on-device: data cols are overwritten by emit_v, the ones columns
        # are memset here, and the unused pad cols only feed never-read PSUM
        # partitions (they still get zeroed by the full memset below so the
        # race checker sees initialized reads).
        # x slab 0 is the critical startup transfer (~136 GB/s per ring):
        # split it sync/scalar so it lands ~2us earlier
        # single-dc chunks: the qs=0 projections chase slab 0's arrival, and
        # finer chunks keep the per-chunk stall under the HAM idle window
        xs_next = xsl.tile([P, DC, 512], BF16, name="xs")
        for c in range(DC):
            nc.sync.dma_start(out=xs_next[:, c:c + 1, :],
                              in_=xt_d[:, 0, c:c + 1, :])
        nc.scalar.dma_start(out=wq_s[:, 0:4], in_=wq_d[:, 0:4])
        nc.scalar.dma_start(out=wq_s[:, 4:8], in_=wq_d[:, 4:8])
        nc.scalar.dma_start(out=wk_s[:, 0:4], in_=wk_d[:, 0:4])
        nc.scalar.dma_start(out=wk_s[:, 4:8], in_=wk_d[:, 4:8])
        nc.scalar.dma_start(out=tri[:], in_=tri_d[:])
        # wv/wo (not needed until ~16us/~25us) are gated behind x slab 0's
        # last chunks with tiny dependency-creating writes, so the startup's
        # chip-HBM bandwidth goes to the critical slab0+wq/wk path first
        nc.gpsimd.tensor_copy(wv_s[0:1, 0, 0:1], xs_next[0:1, 6, 0:1])
        nc.gpsimd.dma_start(out=wv_s[:], in_=wv_d[:])
        nc.gpsimd.tensor_copy(wo2[0:1, 0, 0:1], xs_next[0:1, 7, 0:1])
        nc.gpsimd.dma_start(out=wo2[:], in_=wo_d[:])

        # split the big zero-fills across gpsimd/vector so the vector queue is
        # free for the first qkv evictions as soon as projections land
        nc.gpsimd.memset(kTz0[64:128, :, :], 0.0)
        nc.vector.memset(kTz1[0:64, :, :], 0.0)
        # vp init: ones columns (softmax denominators) + zero pad strips; the
        # data columns are fully written by emit_v before any AV reads them
        vpar = vp[:].rearrange("p tt (hp par) c -> p tt hp par c", par=2)
        nc.vector.memset(vpar[:, :, :, 0, DH:P], 0.0)
        nc.vector.memset(vpar[:, :, :, 1, 0:DH], 0.0)
        nc.vector.memset(vpar[:, :, :, 0, DH:DH + 1], 1.0)
        nc.vector.memset(vpar[:, :, :, 1, 0:1], 1.0)

        # ---- emission helpers ----
        def emit_slab(ts):
            nonlocal xs_next
            xs = xs_next
            if ts + 1 < QS:
                xs_next = xsl.tile([P, DC, 512], BF16, name="xs")
                if ts == 0:
                    # slab 1 split across both HWDGE rings, gated behind
                    # slab 0's completion (same HBM-contention reasoning)
                    nc.gpsimd.tensor_copy(xs_next[0:1, 0, 0:1], xs[0:1, 7, 0:1])
                    nc.gpsimd.tensor_copy(xs_next[0:1, 4, 0:1], xs[0:1, 7, 0:1])
                    nc.sync.dma_start(out=xs_next[:, 0:4], in_=xt_d[:, 1, 0:4])
                    nc.scalar.dma_start(out=xs_next[:, 4:8], in_=xt_d[:, 1, 4:8])
                else:
                    nc.sync.dma_start(out=xs_next[:], in_=xt_d[:, ts + 1])
            sl = slice(ts * 512, (ts + 1) * 512)

            def emit_proj(w_s, which, jb):
                pp = pj_ps.tile([P, 512], F32, name="pp", tag="pj")
                for dc in range(DC):
                    nc.tensor.matmul(
                        out=pp[:],
                        lhsT=w_s[:, dc, jb * P:(jb + 1) * P],
                        rhs=xs[:, dc, :],
                        start=(dc == 0),
                        stop=(dc == DC - 1),
                    )
                if which == "q":
                    nc.vector.tensor_copy(qT[:, jb, sl], pp[:])
                else:
                    nc.vector.tensor_copy(kTz0[0:64, jb, sl], pp[0:64, :])
                    nc.vector.tensor_copy(kTz1[64:128, jb, sl], pp[64:128, :])

            def emit_proj_pair(w_s, which):
                # both jb halves chunk-major: same two pj-ring slots as the
                # sequential form, but each arriving x chunk unblocks 2 MMs,
                # halving the slab-0 chunk-chase gaps that re-throttle HAM
                pp = [pj_ps.tile([P, 512], F32, name="pp", tag="pj")
                      for _ in range(2)]
                for dc in range(DC):
                    for jb in range(2):
                        nc.tensor.matmul(
                            out=pp[jb][:],
                            lhsT=w_s[:, dc, jb * P:(jb + 1) * P],
                            rhs=xs[:, dc, :],
                            start=(dc == 0),
                            stop=(dc == DC - 1),
                        )
                for jb in range(2):
                    if which == "q":
                        nc.vector.tensor_copy(qT[:, jb, sl], pp[jb][:])
                    else:
                        nc.vector.tensor_copy(kTz0[0:64, jb, sl], pp[jb][0:64, :])
                        nc.vector.tensor_copy(kTz1[64:128, jb, sl], pp[jb][64:128, :])

            deferred = []
            if ts == 0:
                emit_proj_pair(wq_s, "q")
                emit_proj_pair(wk_s, "k")
            else:
                for jb in range(2):
                    emit_proj(wq_s, "q", jb)
                for jb in range(2):
                    deferred.append(lambda b=jb: emit_proj(wk_s, "k", b))

            def emit_v(j, tt):
                pv = pj_ps.tile([P, F], F32, name="pv", tag="pj")
                for dc in range(DC):
                    nc.tensor.matmul(
                        out=pv[:],
                        lhsT=xs[:, dc, j * P:(j + 1) * P],
                        rhs=wv_s[:, dc, :],
                        start=(dc == 0),
                        stop=(dc == DC - 1),
                    )
                pvv = pv[:].rearrange("p (hp par dh) -> p hp par dh", hp=2, par=2, dh=DH)
                ve = vp[:, tt, :, :].rearrange("p (hp par) c -> p hp par c", par=2)
                nc.vector.tensor_copy(ve[:, :, 0, 0:DH], pvv[:, :, 0, :])
                nc.vector.tensor_copy(ve[:, :, 1, DH:P], pvv[:, :, 1, :])
            return deferred + [(lambda a=j_, b=tt_: emit_v(a, b)) for j_, tt_ in enumerate(range(4 * ts, 4 * ts + 4))]

        state = {}

        def emit_S(qs, h, kp):
            jbh, par = h // 2, h % 2
            kTz = kTz0 if par == 0 else kTz1
            q0 = qs * 512
            spair = sp_ps.tile([P, 1024], F32, name="spair", tag="sp")
            for half in range(2):
                kt = 2 * kp + half
                k0 = kt * P
                sreg = spair[:, half * 512:(half + 1) * 512]
                lhsk = kTz[:, jbh, k0:k0 + P]
                rhsq = qT[:, jbh, :]
                d = k0 - q0
                if d > 0:
                    nc.tensor.matmul(out=sreg[:, d:512], lhsT=lhsk,
                                     rhs=rhsq[:, q0 + d:q0 + 512],
                                     start=True, stop=True)
                else:
                    nc.tensor.matmul(out=sreg, lhsT=lhsk,
                                     rhs=rhsq[:, q0:q0 + 512],
                                     start=True, stop=True)
            state[(qs, h, kp)] = spair

        def emit_exp(qs, h, kp):
            spair = state[(qs, h, kp)]
            q0 = qs * 512
            pt = ptp.tile([P, 1024], BF16, name="pt")
            d1 = (2 * kp + 1) * P - q0
            if d1 > 0:
                # diagonal pair: exp only the written PSUM regions
                d0 = max(d1 - P, 0)
                nc.scalar.activation(pt[:, d0:512], spair[:, d0:512],
                                     mybir.ActivationFunctionType.Exp, scale=0.125)
                nc.scalar.activation(pt[:, 512 + d1:1024], spair[:, 512 + d1:1024],
                                     mybir.ActivationFunctionType.Exp, scale=0.125)
            else:
                nc.scalar.activation(pt[:], spair[:],
                                     mybir.ActivationFunctionType.Exp, scale=0.125)
            # in-tile causal triangle on diagonal k-tiles
            for half in range(2):
                d = (2 * kp + half) * P - q0
                if d >= 0:
                    c = half * 512 + d
                    nc.vector.tensor_mul(pt[:, c:c + P], pt[:, c:c + P], tri[:])
            state[(qs, h, kp)] = (spair, pt)

        def emit_AV(qs, h, kp, nkt):
            _, pt = state.pop((qs, h, kp))
            q0 = qs * 512
            if kp == 0:
                state[(qs, h)] = o_ps.tile([P, 512], F32, name="opsum")
            opsum = state[(qs, h)]
            for half in range(2):
                kt = 2 * kp + half
                d = max(kt * P - q0, 0)
                nc.tensor.matmul(
                    out=opsum[:, d:512],
                    lhsT=vp[:, kt, h, :],
                    rhs=pt[:, half * 512 + d:half * 512 + 512],
                    start=(kt == 0),
                    stop=(kt == nkt - 1),
                )

        def emit_normA(qs, h):
            # full-tile approx reciprocal straight from PSUM (custom-DVE ops
            # only work at base partition 0); only row r is meaningful
            opsum = state[(qs, h)]
            r = DH if h % 2 == 0 else 0
            invf = stg.tile([P, 512], F32, name="invf")
            nc.vector.reciprocal_approx_fast(invf[:], opsum[:])
            invb = stg.tile([DH + 1, 512], BF16, name="invb")
            nc.vector.tensor_copy(invb[r:r + 1, :], invf[r:r + 1, :])
            state[(qs, h, "inv")] = (invb, r)

        def emit_normB(qs, h, drain=False):
            opsum = state.pop((qs, h))
            invb, r = state.pop((qs, h, "inv"))
            jbh, par = h // 2, h % 2
            q0 = qs * 512
            rows = slice(0, DH) if par == 0 else slice(DH, P)
            if par == 1:
                # denom at partition 0: gpsimd broadcast (full-width out; only
                # base-0 in / base-0 out are implemented in the Q7 ucode)
                bsb = stg.tile([P, 512], BF16, name="bsb")
                nc.gpsimd.partition_broadcast(bsb[:, :], invb[0:1, :])
                nc.vector.tensor_mul(oT[rows, jbh, q0:q0 + 512], opsum[rows, :], bsb[rows, :])
            else:
                # denom at partition 64: rank-1 PE broadcast (verified at base 64).
                # Mid-kernel it rides the sp ring (recycles a long-drained spair
                # slot; pj/o_ps would recycle buffers this normB's own consumers
                # still hold).  In the drain the sp ring holds the pre-opened
                # fc0 y-tile halves, so it rides o_ps there instead (recycling
                # the already-normed previous head's opsum).
                if drain:
                    bcast = o_ps.tile([P, 512], F32, name="opsum")
                else:
                    bcast = sp_ps.tile([P, 512], F32, name="bcast", tag="sp")
                nc.tensor.matmul(out=bcast[:], lhsT=oselB[64:65, 0, :],
                                 rhs=invb[64:65, :], start=True, stop=True)
                bsb = stg.tile([P, 512], BF16, name="bsb")
                nc.vector.tensor_copy(bsb[rows, :], bcast[rows, :])
                nc.vector.tensor_mul(oT[rows, jbh, q0:q0 + 512], opsum[rows, :], bsb[rows, :])

        def emit_ytile(qs, tt, e, drain=False):
            yt = ysb.tile([P, D], BF16, name="yt")
            for jh in range(2):
                yp = pj_ps.tile([P, 512], F32, name="pp", tag="pj")
                for fc in range(2):
                    nc.tensor.matmul(
                        out=yp[:],
                        lhsT=oT[:, fc, tt * P:(tt + 1) * P],
                        rhs=wo2[:, fc, jh * 512:(jh + 1) * 512],
                        start=(fc == 0),
                        stop=(fc == 1),
                    )
                nc.vector.tensor_copy(yt[:, jh * 512:(jh + 1) * 512], yp[:])
                # jh1 dispatch rides gpsimd (SWDGE): the scalar queue is the
                # exp engine and its ~600ns dispatches were adding to the
                # slab-boundary activation backlog
                eng = nc.sync if jh == 0 else nc.gpsimd
                eng.dma_start(out=y_d[tt * P:(tt + 1) * P, jh * 512:(jh + 1) * 512],
                              in_=yt[:, jh * 512:(jh + 1) * 512])

        # ---- fused pipeline ----
        # qs=3 head order [1,0,3,2]: the last-processed head is even-parity,
        # so the final normB on the drain path is the cheap PE-broadcast, and
        # both fc0 heads (0,1) finish early enough to overlap the drain.
        head_order = {qs: list(range(HG)) for qs in range(QS)}
        head_order[QS - 1] = [1, 0, 3, 2]
        last_head = {qs: head_order[qs][-1] for qs in range(QS)}
        steps = []
        first_step_of_qs = {}
        for qs in range(QS):
            first_step_of_qs[len(steps)] = qs
            for h in head_order[qs]:
                nkt = 4 * qs + 4
                for kp in range(nkt // 2):
                    steps.append((qs, h, kp, nkt))

        todo = []

        def flush(i):
            while todo and todo[0][0] <= i:
                todo.pop(0)[1]()

        nsteps = len(steps)
        for i in range(nsteps):
            if i in first_step_of_qs:
                for vj, fn_ in enumerate(emit_slab(first_step_of_qs[i])):
                    todo.append((i + vj, fn_))
                todo.sort(key=lambda e: e[0])
            qs, h, kp, nkt = steps[i]
            emit_S(qs, h, kp)
            flush(i)
            if i >= 1:
                pqs, ph_, pkp, _ = steps[i - 1]
                emit_exp(pqs, ph_, pkp)
            if i >= 2:
                pqs, ph_, pkp, pnkt = steps[i - 2]
                emit_AV(pqs, ph_, pkp, pnkt)
                if pkp == pnkt // 2 - 1:
                    emit_normA(pqs, ph_)
                    todo.append((i + min(5, 2 * (pqs + 1) + 1), lambda q=pqs, hh=ph_: emit_normB(q, hh)))
                    if ph_ == last_head[pqs]:
                        for j, tt in enumerate(range(4 * pqs, 4 * pqs + 4)):
                            todo.append((i + 6 + j,
                                         lambda q=pqs, t_=tt, e=j: emit_ytile(q, t_, e)))
        # drain: last exp/AV, then overlap the final norm chain (even-parity
        # head -> PE broadcast path) with the fc0 halves of the first two
        # output tiles.  Only 2 yp tiles are pre-opened: the 3rd sp_ps buffer
        # must stay free for the normB bcast matmul (a 3rd yp would recycle
        # its buffer and deadlock against the un-evicted fc1 halves).
        emit_exp(*steps[nsteps - 1][:3])
        last_norm = None
        for i in (nsteps - 2, nsteps - 1):
            qs, h, kp, nkt = steps[i]
            emit_AV(qs, h, kp, nkt)
            if kp == nkt // 2 - 1:
                emit_normA(qs, h)
                last_norm = (qs, h)
        for _, fn in todo:
            fn()
        lqs = QS - 1

        # pre-open y tiles 12 (sp ring — spair traffic is over) and 13 (pj
        # ring) and run their fc0 matmuls so the PE stays busy through the
        # final normB chain; the drain normB's bcast rides o_ps instead
        tt0 = 4 * lqs
        pre = []
        with tc.high_priority(offset=300):
            for tt, pool, tag in ((tt0, sp_ps, "sp"), (tt0 + 1, pj_ps, "pj")):
                yps = []
                for jh in range(2):
                    yp = pool.tile([P, 512], F32,
                                   name="bcast" if tag == "sp" else "pp", tag=tag)
                    nc.tensor.matmul(out=yp[:], lhsT=oT[:, 0, tt * P:(tt + 1) * P],
                                     rhs=wo2[:, 0, jh * 512:(jh + 1) * 512],
                                     start=True, stop=False)
                    yps.append(yp)
                pre.append((tt, yps))
        emit_normB(*last_norm, drain=True)
        # drain evictions split DVE / scalar-Copy (the loaded exp table set
        # also serves Copy, so no ACT_TABLE_LOAD) to halve the serial tail
        for tt, yps in pre:
            yt = ysb.tile([P, D], BF16, name="yt")
            for jh in range(2):
                nc.tensor.matmul(out=yps[jh][:],
                                 lhsT=oT[:, 1, tt * P:(tt + 1) * P],
                                 rhs=wo2[:, 1, jh * 512:(jh + 1) * 512],
                                 start=False, stop=True)
                if jh == 0:
                    nc.vector.tensor_copy(yt[:, 0:512], yps[0][:])
                    nc.sync.dma_start(out=y_d[tt * P:(tt + 1) * P, 0:512],
                                      in_=yt[:, 0:512])
                else:
                    nc.scalar.activation(yt[:, 512:D], yps[1][:],
                                         mybir.ActivationFunctionType.Copy)
                    nc.scalar.dma_start(out=y_d[tt * P:(tt + 1) * P, 512:D],
                                        in_=yt[:, 512:D])
        # last two tiles ride the (now idle) o_ps pool so they don't wait for
        # the pj ring to recycle
        for tt in range(tt0 + 2, tt0 + 4):
            yph = [o_ps.tile([P, 512], F32, name="opsum") for _ in range(2)]
            for fc in range(2):
                for jh in range(2):
                    nc.tensor.matmul(
                        out=yph[jh][:],
                        lhsT=oT[:, fc, tt * P:(tt + 1) * P],
                        rhs=wo2[:, fc, jh * 512:(jh + 1) * 512],
                        start=(fc == 0),
                        stop=(fc == 1),
                    )
            yt2 = ysb.tile([P, D], BF16, name="yt")
            nc.vector.tensor_copy(yt2[:, 0:512], yph[0][:])
            nc.sync.dma_start(out=y_d[tt * P:(tt + 1) * P, 0:512],
                              in_=yt2[:, 0:512])
            nc.scalar.activation(yt2[:, 512:D], yph[1][:],
                                 mybir.ActivationFunctionType.Copy)
            nc.scalar.dma_start(out=y_d[tt * P:(tt + 1) * P, 512:D],
                                in_=yt2[:, 512:D])

    nc.compile()
    return nc


def make_core_inputs(x, Wq, Wk, Wv, Wo):
    import ml_dtypes
    bf = ml_dtypes.bfloat16

    tri = np.triu(np.ones((P, P), dtype=np.float32)).astype(bf)

    def pmajor(w, chunks):  # [chunks*P, f] -> [P, chunks, f]
        return np.ascontiguousarray(
            w.reshape(chunks, P, -1).transpose(1, 0, 2)).astype(bf)

    in_maps = []
    for c in range(NCORES):
        b, hg = c // HG, c % HG
        s = slice(hg * F, (hg + 1) * F)
        # x^T slab-major: [D, T] -> [P, QS, DC, 512]
        xt = np.ascontiguousarray(
            x[b].T.reshape(DC, P, QS, 512).transpose(1, 2, 0, 3)).astype(bf)
        in_maps.append({
            "xt": xt,
            "wq": pmajor(Wq[:, s], DC),
            "wk": pmajor(Wk[:, s], DC),
            "wv": pmajor(Wv[:, s], DC),
            "wo": pmajor(Wo[s, :], 2),
            "tri": tri,
        })
    return in_maps


_NC_CACHE = None


def _get_nc():
    global _NC_CACHE
    if _NC_CACHE is None:
        _NC_CACHE = build_nc()
    return _NC_CACHE


def kernel(x, Wq, Wk, Wv, Wo):
    global LAST_RESULTS
    _install_ntff_hook()
    from concourse.bass_utils import run_bass_kernel_spmd

    x = np.asarray(x, dtype=np.float32)
    Wq = np.asarray(Wq, dtype=np.float32)
    Wk = np.asarray(Wk, dtype=np.float32)
    Wv = np.asarray(Wv, dtype=np.float32)
    Wo = np.asarray(Wo, dtype=np.float32)

    nc = _get_nc()
    in_maps = make_core_inputs(x, Wq, Wk, Wv, Wo)
    res = run_bass_kernel_spmd(nc, in_maps, list(range(NCORES)))
    LAST_RESULTS = res

    out = np.zeros((B, T, D), dtype=np.float32)
    for c in range(NCORES):
        out[c // HG] += np.asarray(res.results[c]["y"], dtype=np.float32)
    return out



# revision 64
# speedup vs baseline: 1.0298x; 1.0153x over previous
"""Causal self-attention (B=2, T=2048, D=1024, H=16) on 8 TRN2 NeuronCores.

Sharding: data-parallel over batch (2) x tensor-parallel over head groups (4).
Each core handles 1 batch x 4 heads: Wq/Wk/Wv column-sharded, Wo row-sharded;
each core emits a partial (T, D) output and the host sums 4 partials per batch.

v2 design (vs the fp32r baseline):
  - x is transposed on the HOST and shipped as bf16 [P, QS, DC, 512]
    slab-major, eliminating all 128 PE transposes and their PSUM evictions.
  - All matmul operands are bf16 (fp32 PSUM accumulation): enables the PE's
    fast-weight-load path (fp32r blocks FWL), halves eviction/DMA bytes.
  - No mask-inject matmuls: diagonal S tiles compute only columns [d, 512);
    exp is restricted to the written PSUM region (split calls on diagonal
    k-pairs), and the in-tile causal triangle is applied post-exp as a bf16
    tensor_mul with a host-provided [128,128] upper-triangular mask.
  - All PSUM evictions run on DVE; the scalar engine does (almost) only exp.
  - Softmax denominators ride in V' ones-columns (parity layout: even heads
    col 64 / rows 0-63 data, odd heads col 0 / rows 64-127 data); per (qs,h)
    the denom row is folded via SBUF DMA, reciprocal'd on DVE, unfolded and
    rank-1-broadcast on PE, then fused into the oT eviction multiply.

v3 changes (163.0us -> 152.4us), all pipeline/startup/drain plumbing:
  - V' padding is generated on-device (pad-strip + ones-column memsets)
    instead of DMAing a 2MB host constant; the big zero-fills are split
    gpsimd/vector so the vector queue is free for the first qkv evictions
    (the DVE preamble pile-up was gating the whole qs=0 phase).
  - Startup DMA rings rebalanced: x slab 0 chunked on sync, wq/wk/tri on
    scalar, wv/wo on gpsimd, x slab 1 split across both HWDGE rings (it was
    landing at ~26us on the overloaded scalar ring, stalling qs=1).
  - proj/yout/bcast PSUM moved off the spair ring into a dedicated 2-bank
    ring (their allocations serialized behind spair's exp readers at every
    slab boundary); spair drops to 2 bufs.
  - y DMAs issue per 512-column half on sync+gpsimd queues mid-kernel
    (keeping ~600ns dispatches off the scalar/exp engine) and sync+scalar
    in the drain (where scalar is idle).
  - x slab 0 ships as 8 single-dc chunks; the qs=0 projections chase its
    arrival (~14us, chip-HBM-bound: all 8 cores pull startup data at once)
    and finer chunks keep each stall under the HAM idle window.
  - Drain y tiles 12/13 pre-open their fc0 matmuls on the idle spair/pj
    rings (high_priority) to span the final normB chain; the drain normB
    bcast rides o_ps; tiles 14/15 follow on o_ps.
  - qs=3 runs heads in order [1,0,3,2] so the final normB is the cheap
    PE-broadcast path, and y-tile-12's fc0 matmuls are pre-opened
    (high_priority) to keep the PE warm through the final norm chain.

Explored and rejected (measured on HW via probe kernel): 64-row-tiled S
matmuls give a true 2x when paired back-to-back (108.8 ns per 512-col MM),
but interleaving 64-row and 128-row tile modes costs ~105ns per mode switch,
erasing the win at the per-step granularity PSUM allows (8 banks cap the S
burst depth at 2). fp8 attention fails the 2e-2 gate (~4-12% output noise).
"""

import sys, os, types

sys.path.insert(0, "/opt/trn_rl_repo")

import numpy as np
from contextlib import ExitStack

import concourse.bass as bass
import concourse.mybir as mybir
import concourse.tile as tile
from concourse import bacc

B, T, D, H = 2, 2048, 1024, 16
DH = D // H          # 64
NCORES = 8
HG = 4               # heads per core
F = HG * DH          # 256 local features per core
P = 128
F32 = mybir.dt.float32
F32R = mybir.dt.float32r
BF16 = mybir.dt.bfloat16
FP8 = mybir.dt.float8e4

TT = T // P          # 16 t-tiles
QS = T // 512        # 4 q-slabs
DC = D // P          # 8 d-chunks

LAST_RESULTS = None  # BassKernelResults of the most recent hardware run


def _install_ntff_hook():
    if "antenv.axon_hooks" in sys.modules:
        return
    try:
        import antenv
        from trn_agent_boot.trn_boot import _ntff_profile_via_ctypes

        m = types.ModuleType("antenv.axon_hooks")
        h = _ntff_profile_via_ctypes("/opt/axon/libaxon_pjrt.so")
        m.get_axon_ntff_profile_hook = lambda: h
        m.set_axon_ntff_profile_hook = lambda hh: None
        sys.modules["antenv.axon_hooks"] = m
        antenv.axon_hooks = m
    except Exception:
        pass


def build_nc():
    nc = bacc.Bacc("TRN2", target_bir_lowering=False, debug=False)

    # x^T slab-major: [P, QS, DC, 512] so one q-slab is a contiguous
    # 8KB-per-partition DMA run
    xt_d = nc.dram_tensor("xt", [P, QS, DC, 512], BF16, kind="ExternalInput").ap()
    wq_d = nc.dram_tensor("wq", [P, DC, F], BF16, kind="ExternalInput").ap()
    wk_d = nc.dram_tensor("wk", [P, DC, F], BF16, kind="ExternalInput").ap()
    wv_d = nc.dram_tensor("wv", [P, DC, F], BF16, kind="ExternalInput").ap()
    wo_d = nc.dram_tensor("wo", [P, 2, D], BF16, kind="ExternalInput").ap()
    tri_d = nc.dram_tensor("tri", [P, P], BF16, kind="ExternalInput").ap()
    y_d = nc.dram_tensor("y", [T, D], BF16, kind="ExternalOutput").ap()

    with tile.TileContext(nc) as tc, ExitStack() as ctx:
        const = ctx.enter_context(tc.tile_pool(name="const", bufs=1))
        wpool = ctx.enter_context(tc.tile_pool(name="wpool", bufs=1))
        qkv = ctx.enter_context(tc.tile_pool(name="qkv", bufs=1))
        xsl = ctx.enter_context(tc.tile_pool(name="xsl", bufs=2))
        sp_ps = ctx.enter_context(tc.tile_pool(name="sp_ps", bufs=2, space="PSUM"))
        o_ps = ctx.enter_context(tc.tile_pool(name="o_ps", bufs=2, space="PSUM"))
        # dedicated ring for proj/yout/bcast PSUM so their allocations don't
        # serialize behind spair's exp readers at slab boundaries
        pj_ps = ctx.enter_context(tc.tile_pool(name="pj_ps", bufs=2, space="PSUM"))
        ptp = ctx.enter_context(tc.tile_pool(name="ptp", bufs=4))
        stg = ctx.enter_context(tc.tile_pool(name="stg", bufs=6))
        ysb = ctx.enter_context(tc.tile_pool(name="ysb", bufs=4))

        # ---- constants / warmups ----
        tri = const.tile([P, P], BF16, name="tri")
        # dummy matmul burst: ~4.5us of PE activity during the DMA-bound
        # preamble flips the HAM clock gate to 8/8 before real work arrives
        wsrc = const.tile([P, P], BF16, name="wsrc")
        nc.vector.memset(wsrc[:], 0.0)
        wps = sp_ps.tile([P, 512], F32, name="wps", tag="sp")
        for _ in range(30):
            nc.tensor.matmul(out=wps[:, 0:P], lhsT=wsrc[:], rhs=wsrc[:],
                             start=True, stop=True)

        # NOTE: a DMA-gated "adaptive warmup staircase" was tried here and
        # REGRESSED: HAM only unthrottles on ~3.4us of dense PE activity, and
        # sem-gated dummies trickle too sparsely — the whole early phase ran
        # at half clock (first K=8/8 at 19.5us instead of ~11us).
        # touch Exp early so the ACT table load happens in the idle preamble
        warm_src = const.tile([1, 1], F32, name="warm_src")
        nc.vector.memset(warm_src[:], 0.0)
        warm = const.tile([1, 1], F32, name="warm")
        nc.scalar.activation(warm[:], warm_src[:], mybir.ActivationFunctionType.Exp)
        # touch partition_broadcast early so the gpsimd library IRAM load
        # (~6us) happens in the idle preamble
        wpb = const.tile([P, 64], BF16, name="wpb")
        nc.vector.memset(wpb[0:1, :], 1.0)
        nc.gpsimd.partition_broadcast(wpb[:, :], wpb[0:1, :])
        # denominator-broadcast selectors for the K=1 PE path (base-64 rows):
        # bcast = oselB[r]^T (x) inv[r]; even heads r=64 -> out rows 0:64
        oselB = const.tile([65, 2, P], BF16, name="oselB")
        nc.vector.memset(oselB[0:1, :, :], 0.0)
        nc.vector.memset(oselB[64:65, :, :], 0.0)
        nc.vector.memset(oselB[64:65, 0, 0:64], 1.0)

        # ---- persistent tensors ----
        wq_s = wpool.tile([P, DC, F], BF16, name="wq_s")
        wk_s = wpool.tile([P, DC, F], BF16, name="wk_s")
        wv_s = wpool.tile([P, DC, F], BF16, name="wv_s")
        wo2 = wpool.tile([P, 2, D], BF16, name="wo2")
        qT = qkv.tile([P, 2, T], BF16, name="qT")        # [2 heads x dh, jb, t]
        kTz0 = qkv.tile([P, 2, T], BF16, name="kTz0")    # [k_even; 0]
        kTz1 = qkv.tile([P, 2, T], BF16, name="kTz1")    # [0; k_odd]
        vp = qkv.tile([P, TT, HG, P], BF16, name="vp")   # padded V', parity layouts
        oT = qkv.tile([P, 2, T], BF16, name="oT")        # normalized o^T [f, t]

        # ---- preamble DMAs + fills ----
        # Ring split: x slab 0 on sync (4 chunks for progressive arrival),
        # wq/wk/tri on scalar, wv/wo on gpsimd, x slab 1+ alternate
        # vector/sync.  vp (padded V' with denom ones-columns) is generated
        # on-device: data cols are overwritten by emit_v, the ones columns
        # are memset here, and the unused pad cols only feed never-read PSUM
        # partitions (they still get zeroed by the full memset below so the
        # race checker sees initialized reads).
        # x slab 0 is the critical startup transfer (~136 GB/s per ring):
        # split it sync/scalar so it lands ~2us earlier
        # single-dc chunks: the qs=0 projections chase slab 0's arrival, and
        # finer chunks keep the per-chunk stall under the HAM idle window
        xs_next = xsl.tile([P, DC, 512], BF16, name="xs")
        for c in range(DC):
            nc.sync.dma_start(out=xs_next[:, c:c + 1, :],
                              in_=xt_d[:, 0, c:c + 1, :])
        nc.scalar.dma_start(out=wq_s[:, 0:4], in_=wq_d[:, 0:4])
        nc.scalar.dma_start(out=wq_s[:, 4:8], in_=wq_d[:, 4:8])
        nc.scalar.dma_start(out=wk_s[:, 0:4], in_=wk_d[:, 0:4])
        nc.scalar.dma_start(out=wk_s[:, 4:8], in_=wk_d[:, 4:8])
        nc.scalar.dma_start(out=tri[:], in_=tri_d[:])
        # wv/wo (not needed until ~16us/~25us) are gated behind x slab 0's
        # last chunks with tiny dependency-creating writes, so the startup's
        # chip-HBM bandwidth goes to the critical slab0+wq/wk path first
        nc.gpsimd.tensor_copy(wv_s[0:1, 0, 0:1], xs_next[0:1, 6, 0:1])
        nc.gpsimd.dma_start(out=wv_s[:], in_=wv_d[:])
        nc.gpsimd.tensor_copy(wo2[0:1, 0, 0:1], xs_next[0:1, 7, 0:1])
        nc.gpsimd.dma_start(out=wo2[:], in_=wo_d[:])

        # split the big zero-fills across gpsimd/vector so the vector queue is
        # free for the first qkv evictions as soon as projections land
        nc.gpsimd.memset(kTz0[64:128, :, :], 0.0)
        nc.vector.memset(kTz1[0:64, :, :], 0.0)
        # vp init: ones columns (softmax denominators) + zero pad strips; the
        # data columns are fully written by emit_v before any AV reads them
        vpar = vp[:].rearrange("p tt (hp par) c -> p tt hp par c", par=2)
        nc.vector.memset(vpar[:, :, :, 0, DH:P], 0.0)
        nc.vector.memset(vpar[:, :, :, 1, 0:DH], 0.0)
        nc.vector.memset(vpar[:, :, :, 0, DH:DH + 1], 1.0)
        nc.vector.memset(vpar[:, :, :, 1, 0:1], 1.0)

        # ---- emission helpers ----
        def emit_slab(ts):
            nonlocal xs_next
            xs = xs_next
            if ts + 1 < QS:
                xs_next = xsl.tile([P, DC, 512], BF16, name="xs")
                if ts == 0:
                    # slab 1 split across both HWDGE rings, gated behind
                    # slab 0's completion (same HBM-contention reasoning)
                    nc.gpsimd.tensor_copy(xs_next[0:1, 0, 0:1], xs[0:1, 7, 0:1])
                    nc.gpsimd.tensor_copy(xs_next[0:1, 4, 0:1], xs[0:1, 7, 0:1])
                    nc.sync.dma_start(out=xs_next[:, 0:4], in_=xt_d[:, 1, 0:4])
                    nc.scalar.dma_start(out=xs_next[:, 4:8], in_=xt_d[:, 1, 4:8])
                else:
                    nc.sync.dma_start(out=xs_next[:], in_=xt_d[:, ts + 1])
            sl = slice(ts * 512, (ts + 1) * 512)

            def emit_proj(w_s, which, jb):
                pp = pj_ps.tile([P, 512], F32, name="pp", tag="pj")
                for dc in range(DC):
                    nc.tensor.matmul(
                        out=pp[:],
                        lhsT=w_s[:, dc, jb * P:(jb + 1) * P],
                        rhs=xs[:, dc, :],
                        start=(dc == 0),
                        stop=(dc == DC - 1),
                    )
                if which == "q":
                    nc.vector.tensor_copy(qT[:, jb, sl], pp[:])
                else:
                    nc.vector.tensor_copy(kTz0[0:64, jb, sl], pp[0:64, :])
                    nc.vector.tensor_copy(kTz1[64:128, jb, sl], pp[64:128, :])

            def emit_proj_pair(w_s, which):
                # both jb halves chunk-major: same two pj-ring slots as the
                # sequential form, but each arriving x chunk unblocks 2 MMs,
                # halving the slab-0 chunk-chase gaps that re-throttle HAM
                pp = [pj_ps.tile([P, 512], F32, name="pp", tag="pj")
                      for _ in range(2)]
                for dc in range(DC):
                    for jb in range(2):
                        nc.tensor.matmul(
                            out=pp[jb][:],
                            lhsT=w_s[:, dc, jb * P:(jb + 1) * P],
                            rhs=xs[:, dc, :],
                            start=(dc == 0),
                            stop=(dc == DC - 1),
                        )
                for jb in range(2):
                    if which == "q":
                        nc.vector.tensor_copy(qT[:, jb, sl], pp[jb][:])
                    else:
                        nc.vector.tensor_copy(kTz0[0:64, jb, sl], pp[jb][0:64, :])
                        nc.vector.tensor_copy(kTz1[64:128, jb, sl], pp[jb][64:128, :])

            deferred = []
            if ts == 0:
                emit_proj_pair(wq_s, "q")
                emit_proj_pair(wk_s, "k")
            else:
                for jb in range(2):
                    emit_proj(wq_s, "q", jb)
                for jb in range(2):
                    deferred.append(lambda b=jb: emit_proj(wk_s, "k", b))

            def emit_v(j, tt):
                pv = pj_ps.tile([P, F], F32, name="pv", tag="pj")
                for dc in range(DC):
                    nc.tensor.matmul(
                        out=pv[:],
                        lhsT=xs[:, dc, j * P:(j + 1) * P],
                        rhs=wv_s[:, dc, :],
                        start=(dc == 0),
                        stop=(dc == DC - 1),
                    )
                pvv = pv[:].rearrange("p (hp par dh) -> p hp par dh", hp=2, par=2, dh=DH)
                ve = vp[:, tt, :, :].rearrange("p (hp par) c -> p hp par c", par=2)
                nc.vector.tensor_copy(ve[:, :, 0, 0:DH], pvv[:, :, 0, :])
                nc.vector.tensor_copy(ve[:, :, 1, DH:P], pvv[:, :, 1, :])
            return deferred + [(lambda a=j_, b=tt_: emit_v(a, b)) for j_, tt_ in enumerate(range(4 * ts, 4 * ts + 4))]

        state = {}

        def emit_S(qs, h, kp):
            jbh, par = h // 2, h % 2
            kTz = kTz0 if par == 0 else kTz1
            q0 = qs * 512
            spair = sp_ps.tile([P, 1024], F32, name="spair", tag="sp")
            for half in range(2):
                kt = 2 * kp + half
                k0 = kt * P
                sreg = spair[:, half * 512:(half + 1) * 512]
                lhsk = kTz[:, jbh, k0:k0 + P]
                rhsq = qT[:, jbh, :]
                d = k0 - q0
                if d > 0:
                    nc.tensor.matmul(out=sreg[:, d:512], lhsT=lhsk,
                                     rhs=rhsq[:, q0 + d:q0 + 512],
                                     start=True, stop=True)
                else:
                    nc.tensor.matmul(out=sreg, lhsT=lhsk,
                                     rhs=rhsq[:, q0:q0 + 512],
                                     start=True, stop=True)
            state[(qs, h, kp)] = spair

        def emit_exp(qs, h, kp):
            spair = state[(qs, h, kp)]
            q0 = qs * 512
            pt = ptp.tile([P, 1024], BF16, name="pt")
            d1 = (2 * kp + 1) * P - q0
            if d1 > 0:
                # diagonal pair: exp only the written PSUM regions
                d0 = max(d1 - P, 0)
                nc.scalar.activation(pt[:, d0:512], spair[:, d0:512],
                                     mybir.ActivationFunctionType.Exp, scale=0.125)
                nc.scalar.activation(pt[:, 512 + d1:1024], spair[:, 512 + d1:1024],
                                     mybir.ActivationFunctionType.Exp, scale=0.125)
            else:
                nc.scalar.activation(pt[:], spair[:],
                                     mybir.ActivationFunctionType.Exp, scale=0.125)
            # in-tile causal triangle on diagonal k-tiles
            for half in range(2):
                d = (2 * kp + half) * P - q0
                if d >= 0:
                    c = half * 512 + d
                    nc.vector.tensor_mul(pt[:, c:c + P], pt[:, c:c + P], tri[:])
            state[(qs, h, kp)] = (spair, pt)

        def emit_AV(qs, h, kp, nkt):
            _, pt = state.pop((qs, h, kp))
            q0 = qs * 512
            if kp == 0:
                state[(qs, h)] = o_ps.tile([P, 512], F32, name="opsum")
            opsum = state[(qs, h)]
            for half in range(2):
                kt = 2 * kp + half
                d = max(kt * P - q0, 0)
                nc.tensor.matmul(
                    out=opsum[:, d:512],
                    lhsT=vp[:, kt, h, :],
                    rhs=pt[:, half * 512 + d:half * 512 + 512],
                    start=(kt == 0),
                    stop=(kt == nkt - 1),
                )

        def emit_normA(qs, h):
            # full-tile approx reciprocal straight from PSUM (custom-DVE ops
            # only work at base partition 0); only row r is meaningful
            opsum = state[(qs, h)]
            r = DH if h % 2 == 0 else 0
            invf = stg.tile([P, 512], F32, name="invf")
            nc.vector.reciprocal_approx_fast(invf[:], opsum[:])
            invb = stg.tile([DH + 1, 512], BF16, name="invb")
            nc.vector.tensor_copy(invb[r:r + 1, :], invf[r:r + 1, :])
            state[(qs, h, "inv")] = (invb, r)

        def emit_normB(qs, h, drain=False):
            opsum = state.pop((qs, h))
            invb, r = state.pop((qs, h, "inv"))
            jbh, par = h // 2, h % 2
            q0 = qs * 512
            rows = slice(0, DH) if par == 0 else slice(DH, P)
            if par == 1:
                # denom at partition 0: gpsimd broadcast (full-width out; only
                # base-0 in / base-0 out are implemented in the Q7 ucode)
                bsb = stg.tile([P, 512], BF16, name="bsb")
                nc.gpsimd.partition_broadcast(bsb[:, :], invb[0:1, :])
                nc.vector.tensor_mul(oT[rows, jbh, q0:q0 + 512], opsum[rows, :], bsb[rows, :])
            else:
                # denom at partition 64: rank-1 PE broadcast (verified at base 64).
                # Mid-kernel it rides the sp ring (recycles a long-drained spair
                # slot; pj/o_ps would recycle buffers this normB's own consumers
                # still hold).  In the drain the sp ring holds the pre-opened
                # fc0 y-tile halves, so it rides o_ps there instead (recycling
                # the already-normed previous head's opsum).
                if drain:
                    bcast = o_ps.tile([P, 512], F32, name="opsum")
                else:
                    bcast = sp_ps.tile([P, 512], F32, name="bcast", tag="sp")
                nc.tensor.matmul(out=bcast[:], lhsT=oselB[64:65, 0, :],
                                 rhs=invb[64:65, :], start=True, stop=True)
                bsb = stg.tile([P, 512], BF16, name="bsb")
                if drain:
                    # column-split so the first y tiles' fc1 matmuls unblock
                    # as soon as their half of oT is normalized
                    nc.vector.tensor_copy(bsb[rows, 0:256], bcast[rows, 0:256])
                    nc.vector.tensor_mul(oT[rows, jbh, q0:q0 + 256],
                                         opsum[rows, 0:256], bsb[rows, 0:256])
                    nc.vector.tensor_copy(bsb[rows, 256:512], bcast[rows, 256:512])
                    nc.vector.tensor_mul(oT[rows, jbh, q0 + 256:q0 + 512],
                                         opsum[rows, 256:512], bsb[rows, 256:512])
                else:
                    nc.vector.tensor_copy(bsb[rows, :], bcast[rows, :])
                    nc.vector.tensor_mul(oT[rows, jbh, q0:q0 + 512], opsum[rows, :], bsb[rows, :])

        def emit_ytile(qs, tt, e, drain=False):
            yt = ysb.tile([P, D], BF16, name="yt")
            for jh in range(2):
                yp = pj_ps.tile([P, 512], F32, name="pp", tag="pj")
                for fc in range(2):
                    nc.tensor.matmul(
                        out=yp[:],
                        lhsT=oT[:, fc, tt * P:(tt + 1) * P],
                        rhs=wo2[:, fc, jh * 512:(jh + 1) * 512],
                        start=(fc == 0),
                        stop=(fc == 1),
                    )
                nc.vector.tensor_copy(yt[:, jh * 512:(jh + 1) * 512], yp[:])
                # jh1 dispatch rides gpsimd (SWDGE): the scalar queue is the
                # exp engine and its ~600ns dispatches were adding to the
                # slab-boundary activation backlog
                eng = nc.sync if jh == 0 else nc.gpsimd
                eng.dma_start(out=y_d[tt * P:(tt + 1) * P, jh * 512:(jh + 1) * 512],
                              in_=yt[:, jh * 512:(jh + 1) * 512])

        # ---- fused pipeline ----
        # qs=3 head order [1,0,3,2]: the last-processed head is even-parity,
        # so the final normB on the drain path is the cheap PE-broadcast, and
        # both fc0 heads (0,1) finish early enough to overlap the drain.
        head_order = {qs: list(range(HG)) for qs in range(QS)}
        head_order[QS - 1] = [1, 0, 3, 2]
        last_head = {qs: head_order[qs][-1] for qs in range(QS)}
        steps = []
        first_step_of_qs = {}
        for qs in range(QS):
            first_step_of_qs[len(steps)] = qs
            for h in head_order[qs]:
                nkt = 4 * qs + 4
                for kp in range(nkt // 2):
                    steps.append((qs, h, kp, nkt))

        todo = []

        def flush(i):
            while todo and todo[0][0] <= i:
                todo.pop(0)[1]()

        nsteps = len(steps)
        for i in range(nsteps):
            if i in first_step_of_qs:
                for vj, fn_ in enumerate(emit_slab(first_step_of_qs[i])):
                    todo.append((i + vj, fn_))
                todo.sort(key=lambda e: e[0])
            qs, h, kp, nkt = steps[i]
            emit_S(qs, h, kp)
            flush(i)
            if i >= 1:
                pqs, ph_, pkp, _ = steps[i - 1]
                emit_exp(pqs, ph_, pkp)
            if i >= 2:
                pqs, ph_, pkp, pnkt = steps[i - 2]
                emit_AV(pqs, ph_, pkp, pnkt)
                if pkp == pnkt // 2 - 1:
                    emit_normA(pqs, ph_)
                    todo.append((i + min(5, 2 * (pqs + 1) + 1), lambda q=pqs, hh=ph_: emit_normB(q, hh)))
                    if ph_ == last_head[pqs]:
                        for j, tt in enumerate(range(4 * pqs, 4 * pqs + 4)):
                            todo.append((i + 6 + j,
                                         lambda q=pqs, t_=tt, e=j: emit_ytile(q, t_, e)))
        # drain: last exp/AV, then overlap the final norm chain (even-parity
        # head -> PE broadcast path) with the fc0 halves of the first two
        # output tiles.  Only 2 yp tiles are pre-opened: the 3rd sp_ps buffer
        # must stay free for the normB bcast matmul (a 3rd yp would recycle
        # its buffer and deadlock against the un-evicted fc1 halves).
        emit_exp(*steps[nsteps - 1][:3])
        last_norm = None
        for i in (nsteps - 2, nsteps - 1):
            qs, h, kp, nkt = steps[i]
            emit_AV(qs, h, kp, nkt)
            if kp == nkt // 2 - 1:
                emit_normA(qs, h)
                last_norm = (qs, h)
        for _, fn in todo:
            fn()
        lqs = QS - 1

        # pre-open y tiles 12 (sp ring — spair traffic is over) and 13 (pj
        # ring) and run their fc0 matmuls so the PE stays busy through the
        # final normB chain; the drain normB's bcast rides o_ps instead
        tt0 = 4 * lqs
        pre = []
        with tc.high_priority(offset=300):
            for tt, pool, tag in ((tt0, sp_ps, "sp"), (tt0 + 1, pj_ps, "pj")):
                yps = []
                for jh in range(2):
                    yp = pool.tile([P, 512], F32,
                                   name="bcast" if tag == "sp" else "pp", tag=tag)
                    nc.tensor.matmul(out=yp[:], lhsT=oT[:, 0, tt * P:(tt + 1) * P],
                                     rhs=wo2[:, 0, jh * 512:(jh + 1) * 512],
                                     start=True, stop=False)
                    yps.append(yp)
                pre.append((tt, yps))
        emit_normB(*last_norm, drain=True)
        # drain evictions split DVE / scalar-Copy (the loaded exp table set
        # also serves Copy, so no ACT_TABLE_LOAD) to halve the serial tail
        for tt, yps in pre:
            yt = ysb.tile([P, D], BF16, name="yt")
            for jh in range(2):
                nc.tensor.matmul(out=yps[jh][:],
                                 lhsT=oT[:, 1, tt * P:(tt + 1) * P],
                                 rhs=wo2[:, 1, jh * 512:(jh + 1) * 512],
                                 start=False, stop=True)
                if jh == 0:
                    nc.vector.tensor_copy(yt[:, 0:512], yps[0][:])
                    nc.sync.dma_start(out=y_d[tt * P:(tt + 1) * P, 0:512],
                                      in_=yt[:, 0:512])
                else:
                    nc.scalar.activation(yt[:, 512:D], yps[1][:],
                                         mybir.ActivationFunctionType.Copy)
                    nc.scalar.dma_start(out=y_d[tt * P:(tt + 1) * P, 512:D],
                                        in_=yt[:, 512:D])
        # last two tiles ride the (now idle) o_ps pool so they don't wait for
        # the pj ring to recycle
        # tile 14 rides o_ps (recycling the normed opsums); tile 15 rides the
        # sp ring (recycling tile 12's pre-open slots, which free earliest) so
        # the last two tiles' matmuls aren't serialized by one ring's evictions
        for tt in range(tt0 + 2, tt0 + 4):
            if tt == tt0 + 2:
                yph = [o_ps.tile([P, 512], F32, name="opsum") for _ in range(2)]
            else:
                yph = [sp_ps.tile([P, 512], F32, name="bcast", tag="sp")
                       for _ in range(2)]
            for fc in range(2):
                for jh in range(2):
                    nc.tensor.matmul(
                        out=yph[jh][:],
                        lhsT=oT[:, fc, tt * P:(tt + 1) * P],
                        rhs=wo2[:, fc, jh * 512:(jh + 1) * 512],
                        start=(fc == 0),
                        stop=(fc == 1),
                    )
            yt2 = ysb.tile([P, D], BF16, name="yt")
            nc.vector.tensor_copy(yt2[:, 0:512], yph[0][:])
            nc.sync.dma_start(out=y_d[tt * P:(tt + 1) * P, 0:512],
                              in_=yt2[:, 0:512])
            nc.scalar.activation(yt2[:, 512:D], yph[1][:],
                                 mybir.ActivationFunctionType.Copy)
            nc.scalar.dma_start(out=y_d[tt * P:(tt + 1) * P, 512:D],
                                in_=yt2[:, 512:D])

    nc.compile()
    return nc


def make_core_inputs(x, Wq, Wk, Wv, Wo):
    import ml_dtypes
    bf = ml_dtypes.bfloat16

    tri = np.triu(np.ones((P, P), dtype=np.float32)).astype(bf)

    def pmajor(w, chunks):  # [chunks*P, f] -> [P, chunks, f]
        return np.ascontiguousarray(
            w.reshape(chunks, P, -1).transpose(1, 0, 2)).astype(bf)

    in_maps = []
    for c in range(NCORES):
        b, hg = c // HG, c % HG
        s = slice(hg * F, (hg + 1) * F)
        # x^T slab-major: [D, T] -> [P, QS, DC, 512]
        xt = np.ascontiguousarray(
            x[b].T.reshape(DC, P, QS, 512).transpose(1, 2, 0, 3)).astype(bf)
        in_maps.append({
            "xt": xt,
            "wq": pmajor(Wq[:, s], DC),
            "wk": pmajor(Wk[:, s], DC),
            "wv": pmajor(Wv[:, s], DC),
            "wo": pmajor(Wo[s, :], 2),
            "tri": tri,
        })
    return in_maps


_NC_CACHE = None


def _get_nc():
    global _NC_CACHE
    if _NC_CACHE is None:
        _NC_CACHE = build_nc()
    return _NC_CACHE


def kernel(x, Wq, Wk, Wv, Wo):
    global LAST_RESULTS
    _install_ntff_hook()
    from concourse.bass_utils import run_bass_kernel_spmd

    x = np.asarray(x, dtype=np.float32)
    Wq = np.asarray(Wq, dtype=np.float32)
    Wk = np.asarray(Wk, dtype=np.float32)
    Wv = np.asarray(Wv, dtype=np.float32)
    Wo = np.asarray(Wo, dtype=np.float32)

    nc = _get_nc()
    in_maps = make_core_inputs(x, Wq, Wk, Wv, Wo)
    res = run_bass_kernel_spmd(nc, in_maps, list(range(NCORES)))
    LAST_RESULTS = res

    out = np.zeros((B, T, D), dtype=np.float32)
    for c in range(NCORES):
        out[c // HG] += np.asarray(res.results[c]["y"], dtype=np.float32)
    return out

